# revision 47
# baseline (speedup 1.0000x reference)
"""Trainium2 Bass kernel for nn_GroupedQueryAttention_86380382257377.

Math: the reference einsums collapse —
  scores[b,q,h,g] = x[b,q,:] . wq_eff[b][:, g, h] + bqdot[b,g,h]
      with wq_eff[b][e,(g,h)] = sum_k Wq[e,(g,h),k] * ksum[b,g,k],
           ksum[b,g] = Wk_g^T xs[b] + S*bk_g,  xs[b] = sum_s x[b,s,:]
  weights = softmax_g(scores);  wsum[b,g] = sum_{q,h} weights
  out[b]  = x[b] @ M[b] + cvec[b],
      M[b] = sum_g wsum[b,g] * (Wv_g @ Wo_g),
      cvec[b] = sum_g wsum[b,g] * (bv_g @ Wo_g) + bo.

Sharding (8 cores): core c owns group c for the Wq/Wk shard (one small
fp16 AllGather of (wq_eff, bqdot)); x is replicated (fp16); the P = Wv@Wo
and x@M stages are column-sharded (64 output cols per core).  All heavy
matmuls and DMAs run in fp16 (PSUM accumulation stays fp32).
"""

import numpy as np

B, S, D, G, H = 2, 2048, 512, 8, 4
N_CORES = 8
FSL = D // N_CORES  # 64 output columns per core
P = 128
DC = D // P   # 4 chunks of the contraction dims
JC = S // P   # 16 score row-chunks
SC = S // 512  # 4 out column-chunks
NXCH = 4      # x DMA chunks (for overlapped xs reduction)
XCW = S // NXCH  # 512
INV_SQRT_D = 1.0 / float(np.sqrt(D))
CHUNK = D * B * H + B * H  # 4096 wq_eff + 8 bqdot  (fp16 elements)

_cache = {}


def _build_nc():
    import concourse.bass as bass
    import concourse.mybir as mybir
    import concourse.tile as tile
    from concourse import bacc

    f32 = mybir.dt.float32
    f16 = mybir.dt.float16
    f8 = mybir.dt.float8e4
    nc = bacc.Bacc(None, num_devices=N_CORES)

    # ---- kernel I/O (host-prepared, fp16 unless noted) ----
    xT_d = nc.dram_tensor("xT16", [D, B, S], f16, kind="ExternalInput")     # [d,b,s]
    wk_d = nc.dram_tensor("wk16", [D, D], f16, kind="ExternalInput")        # [d,k]
    wq_d = nc.dram_tensor("wq16", [D, H, D], f8, kind="ExternalInput")      # [a,h,e]
    wvT_d = nc.dram_tensor("wvT16", [G, D, D], f16, kind="ExternalInput")   # [g,e,d]
    wo_d = nc.dram_tensor("wo16", [D, G, FSL], f16, kind="ExternalInput")   # [e,g,f]
    bq_d = nc.dram_tensor("bq16", [D, H], f16, kind="ExternalInput")        # [k,h]
    bk_d = nc.dram_tensor("bk32", [D], f32, kind="ExternalInput")
    bv_d = nc.dram_tensor("bv32", [D, G], f32, kind="ExternalInput")        # [e,g]
    bo_d = nc.dram_tensor("bo32", [FSL], f32, kind="ExternalInput")
    out_d = nc.dram_tensor("out16", [B, JC, P, FSL], f16, kind="ExternalOutput")
    cv_d = nc.dram_tensor("cvec32", [FSL, B], f32, kind="ExternalOutput")

    with tile.TileContext(nc) as tc:
        with (
            tc.tile_pool(name="sing", bufs=1) as sing,
            tc.tile_pool(name="pps", bufs=1, space="PSUM") as pps,
            tc.tile_pool(name="pp", bufs=2, space="PSUM") as pp,
            tc.tile_pool(name="pss", bufs=2, space="PSUM") as pss,
            tc.tile_pool(name="pws", bufs=1, space="PSUM") as pws,
            tc.tile_pool(name="ppo", bufs=2, space="PSUM") as ppo,
            tc.tile_pool(name="dram", bufs=1, space="DRAM") as dram,
        ):
            # ---- persistent SBUF tiles ----
            x_sb = sing.tile([P, DC, B, S], f16)          # 32KB/part
            red = sing.tile([P, DC, B, 1024], f16)        # xs tree scratch
            wv_sb = sing.tile([P, G, DC, D], f16)         # lhsT [e, d] per (g,ec)
            wq_sb = sing.tile([P, DC, H, D], f8)          # lhsT [a, e] per (h,ac)
            wk_sb = sing.tile([P, DC, D], f16)            # lhsT [d, k] per (kc,dc)
            wo_sb = sing.tile([P, DC, G, FSL], f16)       # rhs [e, f] per (g,ec)
            bq_sb = sing.tile([P, DC, H], f16)            # rhs [k, h]
            bk_sb = sing.tile([P, DC], f32)
            bkS_sb = sing.tile([P, DC], f32)
            bv16 = sing.tile([P, DC, G], f16)
            bvo_sb = sing.tile([FSL, G], f32)
            bo_sb = sing.tile([FSL, 1], f32)
            cvec_sb = sing.tile([FSL, B], f32)
            ones_sb = sing.tile([P, P], f16)
            xs32 = sing.tile([P, DC, B], f32)
            xs16 = sing.tile([P, DC, B], f16)
            ksum16 = sing.tile([P, DC, B], f16)
            wqe_loc = sing.tile([P, DC, B, H], f16)
            bqd_loc = sing.tile([B, H], f16)
            wqe_all = sing.tile([P, G, DC, B, H], f16)
            bqd_all = sing.tile([1, B, G, H], f16)
            s1_sb = sing.tile([P, B, JC, G, H], f32)      # exp(scores)
            den_sb = sing.tile([P, B, JC, H], f32)
            rec_sb = sing.tile([P, B, JC, H], f32)
            w16_sb = sing.tile([P, B, JC, G, H], f16)     # softmax weights
            wsum_sb = sing.tile([1, B, G], f32)
            ws16_sb = sing.tile([1, B * G], f16)
            wsum_bc = sing.tile([P, B, G], f32)
            p16 = sing.tile([P, G, DC, FSL], f16)         # P_g[:, fslice]
            m16 = sing.tile([P, B, DC, FSL], f16)         # M[b][:, fslice]
            out_sb = sing.tile([P, JC, B, FSL], f16)

            # ---- internal DRAM (collective bounce) ----
            wq_bounce = dram.tile([CHUNK], f16)
            wq_gath = dram.tile([G * CHUNK], f16)

            nc.vector.memset(ones_sb[:, :], 1.0)

            # ---- input DMAs, ordered for the critical path:
            #      x chunks (xs tree), wk, wq  ->  AllGather chain
            #      wv, wo, biases              ->  P / cvec path
            nc.sync.dma_start(
                out=wq_sb[:, :, :, :], in_=wq_d.rearrange("(ac p) h e -> p ac h e", p=P)
            )
            for dc in range(DC):
                for hh in range(2):
                    nc.sync.dma_start(
                        out=x_sb[:, dc, :, hh * 1024:(hh + 1) * 1024],
                        in_=xT_d[dc * P:(dc + 1) * P, :, hh * 1024:(hh + 1) * 1024],
                    )
            nc.sync.dma_start(
                out=wk_sb[:, :, :], in_=wk_d.rearrange("(dc p) k -> p dc k", p=P)
            )
            nc.sync.dma_start(
                out=bk_sb[:, :], in_=bk_d.rearrange("(dc p) -> p dc", p=P)
            )
            nc.sync.dma_start(
                out=bq_sb[:, :, :], in_=bq_d.rearrange("(kc p) h -> p kc h", p=P)
            )

            # ---- A. xs[b,d] = sum_s x : fp16 halving tree per (dc, s-half) ----
            for dc in range(DC):
                for hh in range(2):
                    hb = hh * 1024
                    rb = hh * 512
                    nc.vector.tensor_tensor(
                        out=red[:, dc, :, rb:rb + 512],
                        in0=x_sb[:, dc, :, hb:hb + 512],
                        in1=x_sb[:, dc, :, hb + 512:hb + 1024],
                        op=mybir.AluOpType.add,
                    )
                    w = 256
                    while w >= 8:
                        nc.vector.tensor_tensor(
                            out=red[:, dc, :, rb:rb + w],
                            in0=red[:, dc, :, rb:rb + w],
                            in1=red[:, dc, :, rb + w:rb + 2 * w],
                            op=mybir.AluOpType.add,
                        )
                        w //= 2
            nc.vector.tensor_reduce(
                out=xs32[:, :, :],
                in_=red[:, :, :, :].rearrange(
                    "p dc b (hh o) -> p dc b hh o", hh=2
                )[:, :, :, :, 0:8],
                axis=mybir.AxisListType.XY,
                op=mybir.AluOpType.add,
            )
            nc.vector.tensor_copy(xs16[:, :, :], xs32[:, :, :])

            # ---- B. ksumT[k,b] = Wk_c^T xs + S*bk ----
            nc.vector.tensor_scalar_mul(bkS_sb[:, :], bk_sb[:, :], float(S))
            psmall = pps.tile([P, 512], f32, tag="small")
            psum_k = psmall[:, 0:8].rearrange("p (kc b) -> p kc b", kc=DC)
            for kc in range(DC):
                for dc in range(DC):
                    nc.tensor.matmul(
                        psum_k[:, kc, :],
                        lhsT=wk_sb[:, dc, kc * P:(kc + 1) * P],
                        rhs=xs16[:, dc, :],
                        start=(dc == 0),
                        stop=(dc == DC - 1),
                    )
            bk_b = bkS_sb[:, :]
            nc.vector.tensor_tensor(
                out=ksum16[:, :, :],
                in0=psum_k[:, :, :],
                in1=bass.AP(
                    tensor=bk_b.tensor, offset=bk_b.offset,
                    ap=list(bk_b.ap) + [[0, B]],
                ),
                op=mybir.AluOpType.add,
            )

            # ---- C. wq_eff[e,(b)] per (h, ec); bqdot[b,h]; scale; bounce ----
            psum_wq = psmall[:, 8:40].rearrange(
                "p (ec b h) -> p ec b h", ec=DC, b=B
            )
            for h in range(H):
                for ec in range(DC):
                    for kc in range(DC):
                        nc.tensor.matmul(
                            psum_wq[:, ec, :, h],
                            lhsT=wq_sb[:, kc, h, ec * P:(ec + 1) * P],
                            rhs=ksum16[:, kc, :],
                            start=(kc == 0),
                            stop=(kc == DC - 1),
                        )
            psum_bqd = psmall[0:B, 40:44]
            for kc in range(DC):
                nc.tensor.matmul(
                    psum_bqd[:, :],
                    lhsT=ksum16[:, kc, :],
                    rhs=bq_sb[:, kc, :],
                    start=(kc == 0),
                    stop=(kc == DC - 1),
                )
            nc.vector.tensor_scalar_mul(wqe_loc[:, :, :, :], psum_wq[:, :, :, :], INV_SQRT_D)
            nc.vector.tensor_scalar_mul(bqd_loc[:, :], psum_bqd[:, :], INV_SQRT_D)
            nc.sync.dma_start(
                out=wq_bounce[0:D * B * H].rearrange(
                    "(p ac b h) -> p ac b h", p=P, ac=DC, b=B
                ),
                in_=wqe_loc[:, :, :, :],
            )
            nc.sync.dma_start(
                out=wq_bounce[D * B * H:CHUNK].rearrange("(b h) -> b h", b=B),
                in_=bqd_loc[:, :],
            )

            # ---- D2. weight DMAs for the P path (the AllGather bounce slots
            #      between the 1MB chunks) ----
            for gp in range(4):
                nc.sync.dma_start(
                    out=wv_sb[:, 2 * gp:2 * gp + 2, :, :],
                    in_=wvT_d[2 * gp:2 * gp + 2, :, :].rearrange(
                        "g (ec p) d -> p g ec d", p=P
                    ),
                )
            nc.sync.dma_start(
                out=wo_sb[:, :, :, :],
                in_=wo_d.rearrange("(ec p) g f -> p ec g f", p=P),
            )
            nc.gpsimd.dma_start(
                out=bv16[:, :, :], in_=bv_d.rearrange("(ec p) g -> p ec g", p=P)
            )
            nc.sync.dma_start(
                out=bo_sb[:, :], in_=bo_d.rearrange("(f o) -> f o", o=1)
            )

            # ---- D. AllGather of (wq_eff, bqdot), fp16 ----
            nc.gpsimd.collective_compute(
                "AllGather",
                mybir.AluOpType.bypass,
                replica_groups=[list(range(N_CORES))],
                ins=[wq_bounce[:].opt()],
                outs=[wq_gath[:].opt()],
            )

            # ---- E. spread gathered results ----
            gap = wq_gath[:]
            nc.sync.dma_start(
                out=wqe_all[:, :, :, :, :],
                in_=bass.AP(
                    tensor=gap.tensor,
                    offset=gap.offset,
                    ap=[[DC * B * H, P], [CHUNK, G], [1, DC * B * H]],
                ),
            )
            nc.sync.dma_start(
                out=bqd_all[:, :, :, :],
                in_=bass.AP(
                    tensor=gap.tensor,
                    offset=gap.offset + D * B * H,
                    ap=[[0, 1], [H, B], [CHUNK, G], [1, H]],
                ),
            )

            # ---- F. P_g = Wv_g @ Wo_g[:, fsl]  (all groups, f-slice) ----
            for g in range(G):
                psum_p = pp.tile([P, DC, FSL], f32, tag="pp")
                for dc in range(DC):
                    for ec in range(DC):
                        nc.tensor.matmul(
                            psum_p[:, dc, :],
                            lhsT=wv_sb[:, g, ec, dc * P:(dc + 1) * P],
                            rhs=wo_sb[:, ec, g, :],
                            start=(ec == 0),
                            stop=(ec == DC - 1),
                        )
                nc.scalar.activation(
                    out=p16[:, g, :, :],
                    in_=psum_p[:, :, :],
                    func=mybir.ActivationFunctionType.Copy,
                )

            # ---- F2. bvo[f, g] = bv_g @ Wo_g[:, fsl]  (early) ----
            psum_bvo = psmall[0:FSL, 192:200]
            for g in range(G):
                for ec in range(DC):
                    nc.tensor.matmul(
                        psum_bvo[:, g:g + 1],
                        lhsT=wo_sb[:, ec, g, :],
                        rhs=bv16[:, ec, g:g + 1],
                        start=(ec == 0),
                        stop=(ec == DC - 1),
                    )
            nc.vector.tensor_copy(bvo_sb[:, :], psum_bvo[:, :])

            # ---- G. scores + exp + softmax + wsum (full sequence) ----
            for b in range(B):
                psum_s = pss.tile([P, JC, G * H], f32, tag="ps")
                bq_b = bqd_all[:, b, :, :]
                nc.tensor.matmul(
                    psum_s[:, :, :],
                    lhsT=ones_sb[0:1, :],
                    rhs=bass.AP(
                        tensor=bq_b.tensor, offset=bq_b.offset,
                        ap=[list(bq_b.ap[0]), [0, JC]] + list(bq_b.ap[1:]),
                    ),
                    start=True,
                    stop=False,
                )
                for j in range(JC):
                    for dc in range(DC):
                        nc.tensor.matmul(
                            psum_s[:, j, :],
                            lhsT=x_sb[:, dc, b, j * P:(j + 1) * P],
                            rhs=wqe_all[:, :, dc, b, :],
                            start=False,
                            stop=(j == JC - 1 and dc == DC - 1),
                            skip_group_check=True,
                        )
                nc.scalar.activation(
                    out=s1_sb[:, b, :, :, :].rearrange("p j g h -> p j (g h)"),
                    in_=psum_s[:, :, :],
                    func=mybir.ActivationFunctionType.Exp,
                )
                nc.vector.tensor_reduce(
                    out=den_sb[:, b, :, :],
                    in_=s1_sb[:, b, :, :, :].rearrange("p j g h -> p j h g"),
                    axis=mybir.AxisListType.X,
                    op=mybir.AluOpType.add,
                )
                nc.vector.reciprocal(rec_sb[:, b, :, :], den_sb[:, b, :, :])
                rb = rec_sb[:, b, :, :]
                nc.vector.tensor_tensor(
                    out=w16_sb[:, b, :, :, :].rearrange("p j g h -> p j h g"),
                    in0=s1_sb[:, b, :, :, :].rearrange("p j g h -> p j h g"),
                    in1=bass.AP(
                        tensor=rb.tensor,
                        offset=rb.offset,
                        ap=list(rb.ap) + [[0, G]],
                    ),
                    op=mybir.AluOpType.mult,
                )
                psum_ws = pws.tile([P, JC * G * H], f32, tag="ws")
                nc.tensor.matmul(
                    psum_ws[:, :],
                    lhsT=ones_sb[:, :],
                    rhs=w16_sb[:, b, :, :, :],
                    start=True,
                    stop=True,
                )
                nc.vector.tensor_reduce(
                    out=wsum_bc[:, b, :],
                    in_=psum_ws[:, :].rearrange("p (j g h) -> p g j h", j=JC, g=G),
                    axis=mybir.AxisListType.XY,
                    op=mybir.AluOpType.add,
                )

            # ---- H2. PE warm-up fillers: keep the tensor engine busy through
            #      the softmax/combine window so the out matmuls run at full
            #      clock (cheap redundant column-sums into a recycled bank) ----
            for _ in range(32):
                psum_fill = pss.tile([P, JC, G * H], f32, tag="ps")
                nc.tensor.matmul(
                    psum_fill[:, :, :],
                    lhsT=ones_sb[:, :],
                    rhs=w16_sb[:, 0, :, :, :],
                    start=True,
                    stop=True,
                )

            # ---- I. M[b] = sum_g wsum[b,g] * P_g ----
            mh = sing.tile([P, B, DC, FSL], f16)
            for b in range(B):
                nc.vector.tensor_scalar(
                    out=m16[:, b, :, :],
                    in0=p16[:, 0, :, :],
                    scalar1=wsum_bc[:, b, 0:1],
                    scalar2=None,
                    op0=mybir.AluOpType.mult,
                )
                nc.vector.tensor_scalar(
                    out=mh[:, b, :, :],
                    in0=p16[:, 4, :, :],
                    scalar1=wsum_bc[:, b, 4:5],
                    scalar2=None,
                    op0=mybir.AluOpType.mult,
                )
                for g in (1, 2, 3):
                    nc.vector.scalar_tensor_tensor(
                        out=m16[:, b, :, :],
                        in0=p16[:, g, :, :],
                        scalar=wsum_bc[:, b, g:g + 1],
                        in1=m16[:, b, :, :],
                        op0=mybir.AluOpType.mult,
                        op1=mybir.AluOpType.add,
                    )
                    nc.vector.scalar_tensor_tensor(
                        out=mh[:, b, :, :],
                        in0=p16[:, g + 4, :, :],
                        scalar=wsum_bc[:, b, g + 4:g + 5],
                        in1=mh[:, b, :, :],
                        op0=mybir.AluOpType.mult,
                        op1=mybir.AluOpType.add,
                    )
                nc.vector.tensor_tensor(
                    out=m16[:, b, :, :],
                    in0=m16[:, b, :, :],
                    in1=mh[:, b, :, :],
                    op=mybir.AluOpType.add,
                )

            # ---- K. out[b, s, fsl] = x[b] @ M[b] + cvec  (s on partitions) ----
            for b in range(B):
                for hf in range(2):
                    psum_o = ppo.tile([P, 8, FSL], f32, tag="po")
                    for jj in range(8):
                        j = hf * 8 + jj
                        for dc in range(DC):
                            nc.tensor.matmul(
                                psum_o[:, jj, :],
                                lhsT=x_sb[:, dc, b, j * P:(j + 1) * P],
                                rhs=m16[:, b, dc, :],
                                start=(dc == 0),
                                stop=(dc == DC - 1),
                            )
                    if hf == 0:
                        nc.scalar.activation(
                            out=out_sb[:, hf * 8:(hf + 1) * 8, b, :],
                            in_=psum_o[:, :, :],
                            func=mybir.ActivationFunctionType.Identity,
                        )
                    else:
                        nc.vector.tensor_copy(
                            out_sb[:, hf * 8:(hf + 1) * 8, b, :],
                            psum_o[:, :, :],
                        )
                    nc.sync.dma_start(
                        out=out_d[b, hf * 8:(hf + 1) * 8, :, :].rearrange(
                            "j p f -> p j f"
                        ),
                        in_=out_sb[:, hf * 8:(hf + 1) * 8, b, :],
                    )

            # ---- J. cvec[b] = sum_g wsum[b,g]*bvo[:,g] + bo; flip to [1,(b f)] ----
            for b in range(B):
                nc.vector.scalar_tensor_tensor(
                    out=cvec_sb[:, b:b + 1],
                    in0=bvo_sb[:, 0:1],
                    scalar=wsum_bc[0:FSL, b, 0:1],
                    in1=bo_sb[:, :],
                    op0=mybir.AluOpType.mult,
                    op1=mybir.AluOpType.add,
                )
                for g in range(1, G):
                    nc.vector.scalar_tensor_tensor(
                        out=cvec_sb[:, b:b + 1],
                        in0=bvo_sb[:, g:g + 1],
                        scalar=wsum_bc[0:FSL, b, g:g + 1],
                        in1=cvec_sb[:, b:b + 1],
                        op0=mybir.AluOpType.mult,
                        op1=mybir.AluOpType.add,
                    )
            nc.sync.dma_start(out=cv_d[:, :], in_=cvec_sb[:, :])


    nc.compile()
    return nc


def kernel(x, Wq, bq, Wk, bk, Wv, bv, Wo, bo):
    from concourse.bass_utils import run_bass_kernel_spmd

    if "nc" not in _cache:
        _cache["nc"] = _build_nc()
    nc = _cache["nc"]

    f16 = np.float16
    xT16 = np.ascontiguousarray(
        np.asarray(x, np.float32).transpose(2, 0, 1)).astype(f16)  # [d,b,s]
    wq_r = np.asarray(Wq, np.float32).reshape(D, G, H, D)
    wvT16 = np.ascontiguousarray(
        np.asarray(Wv, np.float32).reshape(D, G, D).transpose(1, 2, 0)
    ).astype(f16)                                                   # [g,e,d]
    wo_r = np.asarray(Wo, np.float32).reshape(G, D, D)
    bq_r = np.asarray(bq, np.float32).reshape(G, H, D)
    in_maps = []
    for c in range(N_CORES):
        fs = slice(c * FSL, (c + 1) * FSL)
        in_maps.append({
            "xT16": xT16,
            "wk16": np.ascontiguousarray(
                np.asarray(Wk, np.float32)[:, c * D:(c + 1) * D]).astype(f16),
            "wq16": np.ascontiguousarray(
                wq_r[:, c].transpose(2, 1, 0)).astype(
                    __import__("ml_dtypes").float8_e4m3),            # [a,h,e]
            "wvT16": wvT16,
            "wo16": np.ascontiguousarray(
                wo_r[:, :, fs].transpose(1, 0, 2)).astype(f16),      # [e,g,f]
            "bq16": np.ascontiguousarray(bq_r[c].T).astype(f16),     # [k,h]
            "bk32": np.ascontiguousarray(
                np.asarray(bk, np.float32)[c * D:(c + 1) * D]),
            "bv32": np.ascontiguousarray(
                np.asarray(bv, np.float32).reshape(G, D).T),         # [e,g]
            "bo32": np.ascontiguousarray(np.asarray(bo, np.float32)[fs]),
        })
    res = run_bass_kernel_spmd(nc, in_maps, core_ids=list(range(N_CORES)))
    _cache["last_results"] = res
    full = np.concatenate(
        [r["out16"].reshape(B, S, FSL) for r in res.results], axis=2
    ).astype(np.float32)                              # [B, S, D]
    cvec = np.concatenate(
        [r["cvec32"].T for r in res.results], axis=1
    )                                                 # [B, D]
    return full + cvec[:, None, :]


# revision 48
# speedup vs baseline: 1.0128x; 1.0128x over previous
"""Trainium2 Bass kernel for nn_GroupedQueryAttention_86380382257377.

Math: the reference einsums collapse —
  scores[b,q,h,g] = x[b,q,:] . wq_eff[b][:, g, h] + bqdot[b,g,h]
      with wq_eff[b][e,(g,h)] = sum_k Wq[e,(g,h),k] * ksum[b,g,k],
           ksum[b,g] = Wk_g^T xs[b] + S*bk_g,  xs[b] = sum_s x[b,s,:]
  weights = softmax_g(scores);  wsum[b,g] = sum_{q,h} weights
  out[b]  = x[b] @ M[b] + cvec[b],
      M[b] = sum_g wsum[b,g] * (Wv_g @ Wo_g),
      cvec[b] = sum_g wsum[b,g] * (bv_g @ Wo_g) + bo.

Sharding (8 cores): core c owns group c for the Wq/Wk shard (one small
fp16 AllGather of (wq_eff, bqdot)); x is replicated (fp16); the P = Wv@Wo
and x@M stages are column-sharded (64 output cols per core).  All heavy
matmuls and DMAs run in fp16 (PSUM accumulation stays fp32).
"""

import numpy as np

B, S, D, G, H = 2, 2048, 512, 8, 4
N_CORES = 8
FSL = D // N_CORES  # 64 output columns per core
P = 128
DC = D // P   # 4 chunks of the contraction dims
JC = S // P   # 16 score row-chunks
SC = S // 512  # 4 out column-chunks
NXCH = 4      # x DMA chunks (for overlapped xs reduction)
XCW = S // NXCH  # 512
INV_SQRT_D = 1.0 / float(np.sqrt(D))
CHUNK = D * B * H + B * H  # 4096 wq_eff + 8 bqdot  (fp16 elements)

_cache = {}


def _build_nc():
    import concourse.bass as bass
    import concourse.mybir as mybir
    import concourse.tile as tile
    from concourse import bacc

    f32 = mybir.dt.float32
    f16 = mybir.dt.float16
    f8 = mybir.dt.float8e4
    nc = bacc.Bacc(None, num_devices=N_CORES)

    # ---- kernel I/O (host-prepared, fp16 unless noted) ----
    xT_d = nc.dram_tensor("xT16", [D, B, S], f16, kind="ExternalInput")     # [d,b,s]
    wk_d = nc.dram_tensor("wk16", [D, D], f16, kind="ExternalInput")        # [d,k]
    wq_d = nc.dram_tensor("wq16", [D, H, D], f8, kind="ExternalInput")      # [a,h,e]
    wvT_d = nc.dram_tensor("wvT16", [G, D, D], f16, kind="ExternalInput")   # [g,e,d]
    wo_d = nc.dram_tensor("wo16", [D, G, FSL], f16, kind="ExternalInput")   # [e,g,f]
    bq_d = nc.dram_tensor("bq16", [D, H], f16, kind="ExternalInput")        # [k,h]
    bk_d = nc.dram_tensor("bk32", [D], f32, kind="ExternalInput")
    bv_d = nc.dram_tensor("bv32", [D, G], f32, kind="ExternalInput")        # [e,g]
    bo_d = nc.dram_tensor("bo32", [FSL], f32, kind="ExternalInput")
    out_d = nc.dram_tensor("out16", [B, JC, P, FSL], f16, kind="ExternalOutput")
    cv_d = nc.dram_tensor("cvec32", [FSL, B], f32, kind="ExternalOutput")

    with tile.TileContext(nc) as tc:
        with (
            tc.tile_pool(name="sing", bufs=1) as sing,
            tc.tile_pool(name="pps", bufs=1, space="PSUM") as pps,
            tc.tile_pool(name="pp", bufs=2, space="PSUM") as pp,
            tc.tile_pool(name="pss", bufs=2, space="PSUM") as pss,
            tc.tile_pool(name="pws", bufs=1, space="PSUM") as pws,
            tc.tile_pool(name="ppo", bufs=2, space="PSUM") as ppo,
            tc.tile_pool(name="dram", bufs=1, space="DRAM") as dram,
        ):
            # ---- persistent SBUF tiles ----
            x_sb = sing.tile([P, DC, B, S], f16)          # 32KB/part
            red = sing.tile([P, DC, B, 1024], f16)        # xs tree scratch
            wv_sb = sing.tile([P, G, DC, D], f16)         # lhsT [e, d] per (g,ec)
            wq_sb = sing.tile([P, DC, H, D], f8)          # lhsT [a, e] per (h,ac)
            wk_sb = sing.tile([P, DC, D], f16)            # lhsT [d, k] per (kc,dc)
            wo_sb = sing.tile([P, DC, G, FSL], f16)       # rhs [e, f] per (g,ec)
            bq_sb = sing.tile([P, DC, H], f16)            # rhs [k, h]
            bk_sb = sing.tile([P, DC], f32)
            bkS_sb = sing.tile([P, DC], f32)
            bv16 = sing.tile([P, DC, G], f16)
            bvo_sb = sing.tile([FSL, G], f32)
            bo_sb = sing.tile([FSL, 1], f32)
            cvec_sb = sing.tile([FSL, B], f32)
            ones_sb = sing.tile([P, P], f16)
            xs32 = sing.tile([P, DC, B], f32)
            xs16 = sing.tile([P, DC, B], f16)
            ksum16 = sing.tile([P, DC, B], f16)
            wqe_loc = sing.tile([P, DC, B, H], f8)
            bqd_loc = sing.tile([B, H], f8)
            wqe_all = sing.tile([P, G, DC, B, H], f8)
            bqd_all = sing.tile([1, B, G, H], f8)
            s1_sb = sing.tile([P, B, JC, G, H], f32)      # exp(scores)
            den_sb = sing.tile([P, B, JC, H], f32)
            rec_sb = sing.tile([P, B, JC, H], f32)
            w16_sb = sing.tile([P, B, JC, G, H], f16)     # softmax weights
            wsum_sb = sing.tile([1, B, G], f32)
            ws16_sb = sing.tile([1, B * G], f16)
            wsum_bc = sing.tile([P, B, G], f32)
            p16 = sing.tile([P, G, DC, FSL], f16)         # P_g[:, fslice]
            m16 = sing.tile([P, B, DC, FSL], f16)         # M[b][:, fslice]
            out_sb = sing.tile([P, JC, B, FSL], f16)

            # ---- internal DRAM (collective bounce) ----
            wq_bounce = dram.tile([CHUNK], f8)
            wq_gath = dram.tile([G * CHUNK], f8)

            nc.vector.memset(ones_sb[:, :], 1.0)

            # ---- input DMAs, ordered for the critical path:
            #      x chunks (xs tree), wk, wq  ->  AllGather chain
            #      wv, wo, biases              ->  P / cvec path
            nc.sync.dma_start(
                out=wq_sb[:, :, :, :], in_=wq_d.rearrange("(ac p) h e -> p ac h e", p=P)
            )
            for dc in range(DC):
                for hh in range(2):
                    nc.sync.dma_start(
                        out=x_sb[:, dc, :, hh * 1024:(hh + 1) * 1024],
                        in_=xT_d[dc * P:(dc + 1) * P, :, hh * 1024:(hh + 1) * 1024],
                    )
            nc.sync.dma_start(
                out=wk_sb[:, :, :], in_=wk_d.rearrange("(dc p) k -> p dc k", p=P)
            )
            nc.sync.dma_start(
                out=bk_sb[:, :], in_=bk_d.rearrange("(dc p) -> p dc", p=P)
            )
            nc.sync.dma_start(
                out=bq_sb[:, :, :], in_=bq_d.rearrange("(kc p) h -> p kc h", p=P)
            )

            # ---- A. xs[b,d] = sum_s x : fp16 halving tree per (dc, s-half) ----
            for dc in range(DC):
                for hh in range(2):
                    hb = hh * 1024
                    rb = hh * 512
                    nc.vector.tensor_tensor(
                        out=red[:, dc, :, rb:rb + 512],
                        in0=x_sb[:, dc, :, hb:hb + 512],
                        in1=x_sb[:, dc, :, hb + 512:hb + 1024],
                        op=mybir.AluOpType.add,
                    )
                    w = 256
                    while w >= 8:
                        nc.vector.tensor_tensor(
                            out=red[:, dc, :, rb:rb + w],
                            in0=red[:, dc, :, rb:rb + w],
                            in1=red[:, dc, :, rb + w:rb + 2 * w],
                            op=mybir.AluOpType.add,
                        )
                        w //= 2
            nc.vector.tensor_reduce(
                out=xs32[:, :, :],
                in_=red[:, :, :, :].rearrange(
                    "p dc b (hh o) -> p dc b hh o", hh=2
                )[:, :, :, :, 0:8],
                axis=mybir.AxisListType.XY,
                op=mybir.AluOpType.add,
            )
            nc.vector.tensor_copy(xs16[:, :, :], xs32[:, :, :])

            # ---- B. ksumT[k,b] = Wk_c^T xs + S*bk ----
            nc.vector.tensor_scalar_mul(bkS_sb[:, :], bk_sb[:, :], float(S))
            psmall = pps.tile([P, 512], f32, tag="small")
            psum_k = psmall[:, 0:8].rearrange("p (kc b) -> p kc b", kc=DC)
            for kc in range(DC):
                for dc in range(DC):
                    nc.tensor.matmul(
                        psum_k[:, kc, :],
                        lhsT=wk_sb[:, dc, kc * P:(kc + 1) * P],
                        rhs=xs16[:, dc, :],
                        start=(dc == 0),
                        stop=(dc == DC - 1),
                    )
            bk_b = bkS_sb[:, :]
            nc.vector.tensor_tensor(
                out=ksum16[:, :, :],
                in0=psum_k[:, :, :],
                in1=bass.AP(
                    tensor=bk_b.tensor, offset=bk_b.offset,
                    ap=list(bk_b.ap) + [[0, B]],
                ),
                op=mybir.AluOpType.add,
            )

            # ---- C. wq_eff[e,(b)] per (h, ec); bqdot[b,h]; scale; bounce ----
            psum_wq = psmall[:, 8:40].rearrange(
                "p (ec b h) -> p ec b h", ec=DC, b=B
            )
            for h in range(H):
                for ec in range(DC):
                    for kc in range(DC):
                        nc.tensor.matmul(
                            psum_wq[:, ec, :, h],
                            lhsT=wq_sb[:, kc, h, ec * P:(ec + 1) * P],
                            rhs=ksum16[:, kc, :],
                            start=(kc == 0),
                            stop=(kc == DC - 1),
                        )
            psum_bqd = psmall[0:B, 40:44]
            for kc in range(DC):
                nc.tensor.matmul(
                    psum_bqd[:, :],
                    lhsT=ksum16[:, kc, :],
                    rhs=bq_sb[:, kc, :],
                    start=(kc == 0),
                    stop=(kc == DC - 1),
                )
            nc.vector.tensor_scalar_mul(wqe_loc[:, :, :, :], psum_wq[:, :, :, :], INV_SQRT_D)
            nc.vector.tensor_scalar_mul(bqd_loc[:, :], psum_bqd[:, :], INV_SQRT_D)
            nc.sync.dma_start(
                out=wq_bounce[0:D * B * H].rearrange(
                    "(p ac b h) -> p ac b h", p=P, ac=DC, b=B
                ),
                in_=wqe_loc[:, :, :, :],
            )
            nc.sync.dma_start(
                out=wq_bounce[D * B * H:CHUNK].rearrange("(b h) -> b h", b=B),
                in_=bqd_loc[:, :],
            )

            # ---- D2. weight DMAs for the P path (the AllGather bounce slots
            #      between the 1MB chunks) ----
            for gp in range(4):
                nc.sync.dma_start(
                    out=wv_sb[:, 2 * gp:2 * gp + 2, :, :],
                    in_=wvT_d[2 * gp:2 * gp + 2, :, :].rearrange(
                        "g (ec p) d -> p g ec d", p=P
                    ),
                )
            nc.sync.dma_start(
                out=wo_sb[:, :, :, :],
                in_=wo_d.rearrange("(ec p) g f -> p ec g f", p=P),
            )
            nc.gpsimd.dma_start(
                out=bv16[:, :, :], in_=bv_d.rearrange("(ec p) g -> p ec g", p=P)
            )
            nc.sync.dma_start(
                out=bo_sb[:, :], in_=bo_d.rearrange("(f o) -> f o", o=1)
            )

            # ---- D. AllGather of (wq_eff, bqdot), fp16 ----
            nc.gpsimd.collective_compute(
                "AllGather",
                mybir.AluOpType.bypass,
                replica_groups=[list(range(N_CORES))],
                ins=[wq_bounce[:].opt()],
                outs=[wq_gath[:].opt()],
            )

            # ---- E. spread gathered results ----
            gap = wq_gath[:]
            nc.sync.dma_start(
                out=wqe_all[:, :, :, :, :],
                in_=bass.AP(
                    tensor=gap.tensor,
                    offset=gap.offset,
                    ap=[[DC * B * H, P], [CHUNK, G], [1, DC * B * H]],
                ),
            )
            nc.sync.dma_start(
                out=bqd_all[:, :, :, :],
                in_=bass.AP(
                    tensor=gap.tensor,
                    offset=gap.offset + D * B * H,
                    ap=[[0, 1], [H, B], [CHUNK, G], [1, H]],
                ),
            )

            # ---- F. P_g = Wv_g @ Wo_g[:, fsl]  (all groups, f-slice) ----
            for g in range(G):
                psum_p = pp.tile([P, DC, FSL], f32, tag="pp")
                for dc in range(DC):
                    for ec in range(DC):
                        nc.tensor.matmul(
                            psum_p[:, dc, :],
                            lhsT=wv_sb[:, g, ec, dc * P:(dc + 1) * P],
                            rhs=wo_sb[:, ec, g, :],
                            start=(ec == 0),
                            stop=(ec == DC - 1),
                        )
                nc.scalar.activation(
                    out=p16[:, g, :, :],
                    in_=psum_p[:, :, :],
                    func=mybir.ActivationFunctionType.Copy,
                )

            # ---- F2. bvo[f, g] = bv_g @ Wo_g[:, fsl]  (early) ----
            psum_bvo = psmall[0:FSL, 192:200]
            for g in range(G):
                for ec in range(DC):
                    nc.tensor.matmul(
                        psum_bvo[:, g:g + 1],
                        lhsT=wo_sb[:, ec, g, :],
                        rhs=bv16[:, ec, g:g + 1],
                        start=(ec == 0),
                        stop=(ec == DC - 1),
                    )
            nc.vector.tensor_copy(bvo_sb[:, :], psum_bvo[:, :])

            # ---- G. scores + exp + softmax + wsum (full sequence) ----
            for b in range(B):
                psum_s = pss.tile([P, JC, G * H], f32, tag="ps")
                bq_b = bqd_all[:, b, :, :]
                nc.tensor.matmul(
                    psum_s[:, :, :],
                    lhsT=ones_sb[0:1, :],
                    rhs=bass.AP(
                        tensor=bq_b.tensor, offset=bq_b.offset,
                        ap=[list(bq_b.ap[0]), [0, JC]] + list(bq_b.ap[1:]),
                    ),
                    start=True,
                    stop=False,
                )
                for j in range(JC):
                    for dc in range(DC):
                        nc.tensor.matmul(
                            psum_s[:, j, :],
                            lhsT=x_sb[:, dc, b, j * P:(j + 1) * P],
                            rhs=wqe_all[:, :, dc, b, :],
                            start=False,
                            stop=(j == JC - 1 and dc == DC - 1),
                            skip_group_check=True,
                        )
                nc.scalar.activation(
                    out=s1_sb[:, b, :, :, :].rearrange("p j g h -> p j (g h)"),
                    in_=psum_s[:, :, :],
                    func=mybir.ActivationFunctionType.Exp,
                )
                nc.vector.tensor_reduce(
                    out=den_sb[:, b, :, :],
                    in_=s1_sb[:, b, :, :, :].rearrange("p j g h -> p j h g"),
                    axis=mybir.AxisListType.X,
                    op=mybir.AluOpType.add,
                )
                nc.vector.reciprocal(rec_sb[:, b, :, :], den_sb[:, b, :, :])
                rb = rec_sb[:, b, :, :]
                nc.vector.tensor_tensor(
                    out=w16_sb[:, b, :, :, :].rearrange("p j g h -> p j h g"),
                    in0=s1_sb[:, b, :, :, :].rearrange("p j g h -> p j h g"),
                    in1=bass.AP(
                        tensor=rb.tensor,
                        offset=rb.offset,
                        ap=list(rb.ap) + [[0, G]],
                    ),
                    op=mybir.AluOpType.mult,
                )
                psum_ws = pws.tile([P, JC * G * H], f32, tag="ws")
                nc.tensor.matmul(
                    psum_ws[:, :],
                    lhsT=ones_sb[:, :],
                    rhs=w16_sb[:, b, :, :, :],
                    start=True,
                    stop=True,
                )
                nc.vector.tensor_reduce(
                    out=wsum_bc[:, b, :],
                    in_=psum_ws[:, :].rearrange("p (j g h) -> p g j h", j=JC, g=G),
                    axis=mybir.AxisListType.XY,
                    op=mybir.AluOpType.add,
                )

            # ---- H2. PE warm-up fillers: keep the tensor engine busy through
            #      the softmax/combine window so the out matmuls run at full
            #      clock (cheap redundant column-sums into a recycled bank) ----
            for _ in range(32):
                psum_fill = pss.tile([P, JC, G * H], f32, tag="ps")
                nc.tensor.matmul(
                    psum_fill[:, :, :],
                    lhsT=ones_sb[:, :],
                    rhs=w16_sb[:, 0, :, :, :],
                    start=True,
                    stop=True,
                )

            # ---- I. M[b] = sum_g wsum[b,g] * P_g ----
            mh = sing.tile([P, B, DC, FSL], f16)
            for b in range(B):
                nc.vector.tensor_scalar(
                    out=m16[:, b, :, :],
                    in0=p16[:, 0, :, :],
                    scalar1=wsum_bc[:, b, 0:1],
                    scalar2=None,
                    op0=mybir.AluOpType.mult,
                )
                nc.vector.tensor_scalar(
                    out=mh[:, b, :, :],
                    in0=p16[:, 4, :, :],
                    scalar1=wsum_bc[:, b, 4:5],
                    scalar2=None,
                    op0=mybir.AluOpType.mult,
                )
                for g in (1, 2, 3):
                    nc.vector.scalar_tensor_tensor(
                        out=m16[:, b, :, :],
                        in0=p16[:, g, :, :],
                        scalar=wsum_bc[:, b, g:g + 1],
                        in1=m16[:, b, :, :],
                        op0=mybir.AluOpType.mult,
                        op1=mybir.AluOpType.add,
                    )
                    nc.vector.scalar_tensor_tensor(
                        out=mh[:, b, :, :],
                        in0=p16[:, g + 4, :, :],
                        scalar=wsum_bc[:, b, g + 4:g + 5],
                        in1=mh[:, b, :, :],
                        op0=mybir.AluOpType.mult,
                        op1=mybir.AluOpType.add,
                    )
                nc.vector.tensor_tensor(
                    out=m16[:, b, :, :],
                    in0=m16[:, b, :, :],
                    in1=mh[:, b, :, :],
                    op=mybir.AluOpType.add,
                )

            # ---- K. out[b, s, fsl] = x[b] @ M[b] + cvec  (s on partitions) ----
            for b in range(B):
                for hf in range(2):
                    psum_o = ppo.tile([P, 8, FSL], f32, tag="po")
                    for jj in range(8):
                        j = hf * 8 + jj
                        for dc in range(DC):
                            nc.tensor.matmul(
                                psum_o[:, jj, :],
                                lhsT=x_sb[:, dc, b, j * P:(j + 1) * P],
                                rhs=m16[:, b, dc, :],
                                start=(dc == 0),
                                stop=(dc == DC - 1),
                            )
                    if hf == 0:
                        nc.scalar.activation(
                            out=out_sb[:, hf * 8:(hf + 1) * 8, b, :],
                            in_=psum_o[:, :, :],
                            func=mybir.ActivationFunctionType.Identity,
                        )
                    else:
                        nc.vector.tensor_copy(
                            out_sb[:, hf * 8:(hf + 1) * 8, b, :],
                            psum_o[:, :, :],
                        )
                    nc.sync.dma_start(
                        out=out_d[b, hf * 8:(hf + 1) * 8, :, :].rearrange(
                            "j p f -> p j f"
                        ),
                        in_=out_sb[:, hf * 8:(hf + 1) * 8, b, :],
                    )

            # ---- J. cvec[b] = sum_g wsum[b,g]*bvo[:,g] + bo; flip to [1,(b f)] ----
            for b in range(B):
                nc.vector.scalar_tensor_tensor(
                    out=cvec_sb[:, b:b + 1],
                    in0=bvo_sb[:, 0:1],
                    scalar=wsum_bc[0:FSL, b, 0:1],
                    in1=bo_sb[:, :],
                    op0=mybir.AluOpType.mult,
                    op1=mybir.AluOpType.add,
                )
                for g in range(1, G):
                    nc.vector.scalar_tensor_tensor(
                        out=cvec_sb[:, b:b + 1],
                        in0=bvo_sb[:, g:g + 1],
                        scalar=wsum_bc[0:FSL, b, g:g + 1],
                        in1=cvec_sb[:, b:b + 1],
                        op0=mybir.AluOpType.mult,
                        op1=mybir.AluOpType.add,
                    )
            nc.sync.dma_start(out=cv_d[:, :], in_=cvec_sb[:, :])


    nc.compile()
    return nc


def kernel(x, Wq, bq, Wk, bk, Wv, bv, Wo, bo):
    from concourse.bass_utils import run_bass_kernel_spmd

    if "nc" not in _cache:
        _cache["nc"] = _build_nc()
    nc = _cache["nc"]

    f16 = np.float16
    xT16 = np.ascontiguousarray(
        np.asarray(x, np.float32).transpose(2, 0, 1)).astype(f16)  # [d,b,s]
    wq_r = np.asarray(Wq, np.float32).reshape(D, G, H, D)
    wvT16 = np.ascontiguousarray(
        np.asarray(Wv, np.float32).reshape(D, G, D).transpose(1, 2, 0)
    ).astype(f16)                                                   # [g,e,d]
    wo_r = np.asarray(Wo, np.float32).reshape(G, D, D)
    bq_r = np.asarray(bq, np.float32).reshape(G, H, D)
    in_maps = []
    for c in range(N_CORES):
        fs = slice(c * FSL, (c + 1) * FSL)
        in_maps.append({
            "xT16": xT16,
            "wk16": np.ascontiguousarray(
                np.asarray(Wk, np.float32)[:, c * D:(c + 1) * D]).astype(f16),
            "wq16": np.ascontiguousarray(
                wq_r[:, c].transpose(2, 1, 0)).astype(
                    __import__("ml_dtypes").float8_e4m3),            # [a,h,e]
            "wvT16": wvT16,
            "wo16": np.ascontiguousarray(
                wo_r[:, :, fs].transpose(1, 0, 2)).astype(f16),      # [e,g,f]
            "bq16": np.ascontiguousarray(bq_r[c].T).astype(f16),     # [k,h]
            "bk32": np.ascontiguousarray(
                np.asarray(bk, np.float32)[c * D:(c + 1) * D]),
            "bv32": np.ascontiguousarray(
                np.asarray(bv, np.float32).reshape(G, D).T),         # [e,g]
            "bo32": np.ascontiguousarray(np.asarray(bo, np.float32)[fs]),
        })
    res = run_bass_kernel_spmd(nc, in_maps, core_ids=list(range(N_CORES)))
    _cache["last_results"] = res
    full = np.concatenate(
        [r["out16"].reshape(B, S, FSL) for r in res.results], axis=2
    ).astype(np.float32)                              # [B, S, D]
    cvec = np.concatenate(
        [r["cvec32"].T for r in res.results], axis=1
    )                                                 # [B, D]
    return full + cvec[:, None, :]


# revision 53
# speedup vs baseline: 1.0160x; 1.0031x over previous
"""Trainium2 Bass kernel for nn_GroupedQueryAttention_86380382257377.

Math: the reference einsums collapse —
  scores[b,q,h,g] = x[b,q,:] . wq_eff[b][:, g, h] + bqdot[b,g,h]
      with wq_eff[b][e,(g,h)] = sum_k Wq[e,(g,h),k] * ksum[b,g,k],
           ksum[b,g] = Wk_g^T xs[b] + S*bk_g,  xs[b] = sum_s x[b,s,:]
  weights = softmax_g(scores);  wsum[b,g] = sum_{q,h} weights
  out[b]  = x[b] @ M[b] + cvec[b],
      M[b] = sum_g wsum[b,g] * (Wv_g @ Wo_g),
      cvec[b] = sum_g wsum[b,g] * (bv_g @ Wo_g) + bo.

Sharding (8 cores): core c owns group c for the Wq/Wk shard (one small
fp16 AllGather of (wq_eff, bqdot)); x is replicated (fp16); the P = Wv@Wo
and x@M stages are column-sharded (64 output cols per core).  All heavy
matmuls and DMAs run in fp16 (PSUM accumulation stays fp32).
"""

import numpy as np

B, S, D, G, H = 2, 2048, 512, 8, 4
N_CORES = 8
FSL = D // N_CORES  # 64 output columns per core
P = 128
DC = D // P   # 4 chunks of the contraction dims
JC = S // P   # 16 score row-chunks
SC = S // 512  # 4 out column-chunks
NXCH = 4      # x DMA chunks (for overlapped xs reduction)
XCW = S // NXCH  # 512
INV_SQRT_D = 1.0 / float(np.sqrt(D))
CHUNK = D * B * H + B * H  # 4096 wq_eff + 8 bqdot  (fp16 elements)

_cache = {}


def _build_nc():
    import concourse.bass as bass
    import concourse.mybir as mybir
    import concourse.tile as tile
    from concourse import bacc

    f32 = mybir.dt.float32
    f16 = mybir.dt.float16
    f8 = mybir.dt.float8e4
    nc = bacc.Bacc(None, num_devices=N_CORES)

    # ---- kernel I/O (host-prepared, fp16 unless noted) ----
    xT_d = nc.dram_tensor("xT16", [D, B, S], f16, kind="ExternalInput")     # [d,b,s]
    wk_d = nc.dram_tensor("wk16", [D, D], f16, kind="ExternalInput")        # [d,k]
    wq_d = nc.dram_tensor("wq16", [D, H, D], f8, kind="ExternalInput")      # [a,h,e]
    wvT_d = nc.dram_tensor("wvT16", [G, D, D], f16, kind="ExternalInput")   # [g,e,d]
    wo_d = nc.dram_tensor("wo16", [D, G, FSL], f16, kind="ExternalInput")   # [e,g,f]
    bq_d = nc.dram_tensor("bq16", [D, H], f16, kind="ExternalInput")        # [k,h]
    bk_d = nc.dram_tensor("bk32", [D], f32, kind="ExternalInput")
    bv_d = nc.dram_tensor("bv32", [D, G], f32, kind="ExternalInput")        # [e,g]
    bo_d = nc.dram_tensor("bo32", [FSL], f32, kind="ExternalInput")
    out_d = nc.dram_tensor("out16", [B, JC, P, FSL], f16, kind="ExternalOutput")
    cv_d = nc.dram_tensor("cvec32", [FSL, B], f32, kind="ExternalOutput")

    with tile.TileContext(nc) as tc:
        with (
            tc.tile_pool(name="sing", bufs=1) as sing,
            tc.tile_pool(name="pps", bufs=1, space="PSUM") as pps,
            tc.tile_pool(name="pp", bufs=2, space="PSUM") as pp,
            tc.tile_pool(name="pss", bufs=2, space="PSUM") as pss,
            tc.tile_pool(name="pws", bufs=1, space="PSUM") as pws,
            tc.tile_pool(name="ppo", bufs=2, space="PSUM") as ppo,
            tc.tile_pool(name="dram", bufs=1, space="DRAM") as dram,
        ):
            # ---- persistent SBUF tiles ----
            x_sb = sing.tile([P, DC, B, S], f16)          # 32KB/part
            red = sing.tile([P, DC, B, 1024], f16)        # xs tree scratch
            wv_sb = sing.tile([P, G, DC, D], f16)         # lhsT [e, d] per (g,ec)
            wq_sb = sing.tile([P, DC, H, D], f8)          # lhsT [a, e] per (h,ac)
            wk_sb = sing.tile([P, DC, D], f16)            # lhsT [d, k] per (kc,dc)
            wo_sb = sing.tile([P, DC, G, FSL], f16)       # rhs [e, f] per (g,ec)
            bq_sb = sing.tile([P, DC, H], f16)            # rhs [k, h]
            bk_sb = sing.tile([P, DC], f32)
            bkS_sb = sing.tile([P, DC], f32)
            bv16 = sing.tile([P, DC, G], f16)
            bvo_sb = sing.tile([FSL, G], f32)
            bo_sb = sing.tile([FSL, 1], f32)
            cvec_sb = sing.tile([FSL, B], f32)
            ones_sb = sing.tile([P, P], f16)
            xs32 = sing.tile([P, DC, B], f32)
            xs16 = sing.tile([P, DC, B], f16)
            ksum16 = sing.tile([P, DC, B], f16)
            wqe_loc = sing.tile([P, DC, B, H], f8)
            bqd_loc = sing.tile([B, H], f8)
            wqe_all = sing.tile([P, G, DC, B, H], f8)
            bqd_all = sing.tile([1, B, G, H], f8)
            s1_sb = sing.tile([P, B, JC, G, H], f32)      # exp(scores)
            den_sb = sing.tile([P, B, JC, H], f32)
            rec_sb = sing.tile([P, B, JC, H], f32)
            w16_sb = sing.tile([P, B, JC, G, H], f16)     # softmax weights
            wsum_sb = sing.tile([1, B, G], f32)
            ws16_sb = sing.tile([1, B * G], f16)
            wsum_bc = sing.tile([P, B, G], f32)
            p16 = sing.tile([P, G, DC, FSL], f16)         # P_g[:, fslice]
            m16 = sing.tile([P, B, DC, FSL], f16)         # M[b][:, fslice]
            out_sb = sing.tile([P, JC, B, FSL], f16)

            # ---- internal DRAM (collective bounce) ----
            wq_bounce = dram.tile([CHUNK], f8)
            wq_gath = dram.tile([G * CHUNK], f8)

            nc.vector.memset(ones_sb[:, :], 1.0)

            # ---- input DMAs, ordered for the critical path:
            #      x chunks (xs tree), wk, wq  ->  AllGather chain
            #      wv, wo, biases              ->  P / cvec path
            nc.sync.dma_start(
                out=wq_sb[:, :, :, :], in_=wq_d.rearrange("(ac p) h e -> p ac h e", p=P)
            )
            for dc in range(DC):
                for hh in range(2):
                    nc.sync.dma_start(
                        out=x_sb[:, dc, :, hh * 1024:(hh + 1) * 1024],
                        in_=xT_d[dc * P:(dc + 1) * P, :, hh * 1024:(hh + 1) * 1024],
                    )
            nc.sync.dma_start(
                out=wk_sb[:, :, :], in_=wk_d.rearrange("(dc p) k -> p dc k", p=P)
            )
            nc.sync.dma_start(
                out=bk_sb[:, :], in_=bk_d.rearrange("(dc p) -> p dc", p=P)
            )
            nc.sync.dma_start(
                out=bq_sb[:, :, :], in_=bq_d.rearrange("(kc p) h -> p kc h", p=P)
            )

            # ---- A. xs[b,d] = sum_s x : fp16 halving tree per (dc, s-half) ----
            for dc in range(DC):
                for hh in range(2):
                    hb = hh * 1024
                    rb = hh * 512
                    nc.vector.tensor_tensor(
                        out=red[:, dc, :, rb:rb + 512],
                        in0=x_sb[:, dc, :, hb:hb + 512],
                        in1=x_sb[:, dc, :, hb + 512:hb + 1024],
                        op=mybir.AluOpType.add,
                    )
                    w = 256
                    while w >= 8:
                        nc.vector.tensor_tensor(
                            out=red[:, dc, :, rb:rb + w],
                            in0=red[:, dc, :, rb:rb + w],
                            in1=red[:, dc, :, rb + w:rb + 2 * w],
                            op=mybir.AluOpType.add,
                        )
                        w //= 2
            nc.vector.tensor_reduce(
                out=xs32[:, :, :],
                in_=red[:, :, :, :].rearrange(
                    "p dc b (hh o) -> p dc b hh o", hh=2
                )[:, :, :, :, 0:8],
                axis=mybir.AxisListType.XY,
                op=mybir.AluOpType.add,
            )
            nc.vector.tensor_copy(xs16[:, :, :], xs32[:, :, :])

            # ---- B. ksumT[k,b] = Wk_c^T xs + S*bk ----
            nc.vector.tensor_scalar_mul(bkS_sb[:, :], bk_sb[:, :], float(S))
            psmall = pps.tile([P, 512], f32, tag="small")
            psum_k = psmall[:, 0:8].rearrange("p (kc b) -> p kc b", kc=DC)
            for kc in range(DC):
                for dc in range(DC):
                    nc.tensor.matmul(
                        psum_k[:, kc, :],
                        lhsT=wk_sb[:, dc, kc * P:(kc + 1) * P],
                        rhs=xs16[:, dc, :],
                        start=(dc == 0),
                        stop=(dc == DC - 1),
                    )
            bk_b = bkS_sb[:, :]
            nc.vector.tensor_tensor(
                out=ksum16[:, :, :],
                in0=psum_k[:, :, :],
                in1=bass.AP(
                    tensor=bk_b.tensor, offset=bk_b.offset,
                    ap=list(bk_b.ap) + [[0, B]],
                ),
                op=mybir.AluOpType.add,
            )

            # ---- C. wq_eff[e,(b)] per (h, ec); bqdot[b,h]; scale; bounce ----
            psum_wq = psmall[:, 8:40].rearrange(
                "p (ec b h) -> p ec b h", ec=DC, b=B
            )
            for h in range(H):
                for ec in range(DC):
                    for kc in range(DC):
                        nc.tensor.matmul(
                            psum_wq[:, ec, :, h],
                            lhsT=wq_sb[:, kc, h, ec * P:(ec + 1) * P],
                            rhs=ksum16[:, kc, :],
                            start=(kc == 0),
                            stop=(kc == DC - 1),
                        )
            psum_bqd = psmall[0:B, 40:44]
            for kc in range(DC):
                nc.tensor.matmul(
                    psum_bqd[:, :],
                    lhsT=ksum16[:, kc, :],
                    rhs=bq_sb[:, kc, :],
                    start=(kc == 0),
                    stop=(kc == DC - 1),
                )
            nc.vector.tensor_scalar_mul(wqe_loc[:, :, :, :], psum_wq[:, :, :, :], INV_SQRT_D)
            nc.vector.tensor_scalar_mul(bqd_loc[:, :], psum_bqd[:, :], INV_SQRT_D)
            nc.sync.dma_start(
                out=wq_bounce[0:D * B * H].rearrange(
                    "(p ac b h) -> p ac b h", p=P, ac=DC, b=B
                ),
                in_=wqe_loc[:, :, :, :],
            )
            nc.sync.dma_start(
                out=wq_bounce[D * B * H:CHUNK].rearrange("(b h) -> b h", b=B),
                in_=bqd_loc[:, :],
            )

            # ---- D2. weight DMAs for the P path (the AllGather bounce slots
            #      between the 1MB chunks) ----
            for g in range(G):
                nc.sync.dma_start(
                    out=wv_sb[:, g:g + 1, :, :],
                    in_=wvT_d[g:g + 1, :, :].rearrange(
                        "g (ec p) d -> p g ec d", p=P
                    ),
                )
            nc.sync.dma_start(
                out=wo_sb[:, :, :, :],
                in_=wo_d.rearrange("(ec p) g f -> p ec g f", p=P),
            )
            nc.gpsimd.dma_start(
                out=bv16[:, :, :], in_=bv_d.rearrange("(ec p) g -> p ec g", p=P)
            )
            nc.sync.dma_start(
                out=bo_sb[:, :], in_=bo_d.rearrange("(f o) -> f o", o=1)
            )

            # ---- D. AllGather of (wq_eff, bqdot), fp16 ----
            nc.gpsimd.collective_compute(
                "AllGather",
                mybir.AluOpType.bypass,
                replica_groups=[list(range(N_CORES))],
                ins=[wq_bounce[:].opt()],
                outs=[wq_gath[:].opt()],
            )

            # ---- E. spread gathered results ----
            gap = wq_gath[:]
            nc.sync.dma_start(
                out=wqe_all[:, :, :, :, :],
                in_=bass.AP(
                    tensor=gap.tensor,
                    offset=gap.offset,
                    ap=[[DC * B * H, P], [CHUNK, G], [1, DC * B * H]],
                ),
            )
            nc.sync.dma_start(
                out=bqd_all[:, :, :, :],
                in_=bass.AP(
                    tensor=gap.tensor,
                    offset=gap.offset + D * B * H,
                    ap=[[0, 1], [H, B], [CHUNK, G], [1, H]],
                ),
            )

            # ---- F. P_g = Wv_g @ Wo_g[:, fsl]  (all groups, f-slice) ----
            for g in range(G):
                psum_p = pp.tile([P, DC, FSL], f32, tag="pp")
                for dc in range(DC):
                    for ec in range(DC):
                        nc.tensor.matmul(
                            psum_p[:, dc, :],
                            lhsT=wv_sb[:, g, ec, dc * P:(dc + 1) * P],
                            rhs=wo_sb[:, ec, g, :],
                            start=(ec == 0),
                            stop=(ec == DC - 1),
                        )
                nc.scalar.activation(
                    out=p16[:, g, :, :],
                    in_=psum_p[:, :, :],
                    func=mybir.ActivationFunctionType.Copy,
                )

            # ---- F2. bvo[f, g] = bv_g @ Wo_g[:, fsl]  (early) ----
            psum_bvo = psmall[0:FSL, 192:200]
            for g in range(G):
                for ec in range(DC):
                    nc.tensor.matmul(
                        psum_bvo[:, g:g + 1],
                        lhsT=wo_sb[:, ec, g, :],
                        rhs=bv16[:, ec, g:g + 1],
                        start=(ec == 0),
                        stop=(ec == DC - 1),
                    )
            nc.vector.tensor_copy(bvo_sb[:, :], psum_bvo[:, :])

            # ---- G. scores + exp + softmax + wsum (full sequence) ----
            for b in range(B):
                psum_s = pss.tile([P, JC, G * H], f32, tag="ps")
                bq_b = bqd_all[:, b, :, :]
                nc.tensor.matmul(
                    psum_s[:, :, :],
                    lhsT=ones_sb[0:1, :],
                    rhs=bass.AP(
                        tensor=bq_b.tensor, offset=bq_b.offset,
                        ap=[list(bq_b.ap[0]), [0, JC]] + list(bq_b.ap[1:]),
                    ),
                    start=True,
                    stop=False,
                )
                for j in range(JC):
                    for dc in range(DC):
                        nc.tensor.matmul(
                            psum_s[:, j, :],
                            lhsT=x_sb[:, dc, b, j * P:(j + 1) * P],
                            rhs=wqe_all[:, :, dc, b, :],
                            start=False,
                            stop=(j == JC - 1 and dc == DC - 1),
                            skip_group_check=True,
                        )
                nc.scalar.activation(
                    out=s1_sb[:, b, :, :, :].rearrange("p j g h -> p j (g h)"),
                    in_=psum_s[:, :, :],
                    func=mybir.ActivationFunctionType.Exp,
                )
                nc.vector.tensor_reduce(
                    out=den_sb[:, b, :, :],
                    in_=s1_sb[:, b, :, :, :].rearrange("p j g h -> p j h g"),
                    axis=mybir.AxisListType.X,
                    op=mybir.AluOpType.add,
                )
                nc.vector.reciprocal(rec_sb[:, b, :, :], den_sb[:, b, :, :])
                rb = rec_sb[:, b, :, :]
                nc.vector.tensor_tensor(
                    out=w16_sb[:, b, :, :, :].rearrange("p j g h -> p j h g"),
                    in0=s1_sb[:, b, :, :, :].rearrange("p j g h -> p j h g"),
                    in1=bass.AP(
                        tensor=rb.tensor,
                        offset=rb.offset,
                        ap=list(rb.ap) + [[0, G]],
                    ),
                    op=mybir.AluOpType.mult,
                )
                psum_ws = pws.tile([P, JC * G * H], f32, tag="ws")
                nc.tensor.matmul(
                    psum_ws[:, :],
                    lhsT=ones_sb[:, :],
                    rhs=w16_sb[:, b, :, :, :],
                    start=True,
                    stop=True,
                )
                nc.vector.tensor_reduce(
                    out=wsum_bc[:, b, :],
                    in_=psum_ws[:, :].rearrange("p (j g h) -> p g j h", j=JC, g=G),
                    axis=mybir.AxisListType.XY,
                    op=mybir.AluOpType.add,
                )

            # ---- H2. PE warm-up fillers: keep the tensor engine busy through
            #      the softmax/combine window so the out matmuls run at full
            #      clock (cheap redundant column-sums into a recycled bank) ----
            for _ in range(32):
                psum_fill = pss.tile([P, JC, G * H], f32, tag="ps")
                nc.tensor.matmul(
                    psum_fill[:, :, :],
                    lhsT=ones_sb[:, :],
                    rhs=w16_sb[:, 0, :, :, :],
                    start=True,
                    stop=True,
                )

            # ---- I. M[b] = sum_g wsum[b,g] * P_g ----
            mh = sing.tile([P, B, DC, FSL], f16)
            for b in range(B):
                nc.vector.tensor_scalar(
                    out=m16[:, b, :, :],
                    in0=p16[:, 0, :, :],
                    scalar1=wsum_bc[:, b, 0:1],
                    scalar2=None,
                    op0=mybir.AluOpType.mult,
                )
                nc.vector.tensor_scalar(
                    out=mh[:, b, :, :],
                    in0=p16[:, 4, :, :],
                    scalar1=wsum_bc[:, b, 4:5],
                    scalar2=None,
                    op0=mybir.AluOpType.mult,
                )
                for g in (1, 2, 3):
                    nc.vector.scalar_tensor_tensor(
                        out=m16[:, b, :, :],
                        in0=p16[:, g, :, :],
                        scalar=wsum_bc[:, b, g:g + 1],
                        in1=m16[:, b, :, :],
                        op0=mybir.AluOpType.mult,
                        op1=mybir.AluOpType.add,
                    )
                    nc.vector.scalar_tensor_tensor(
                        out=mh[:, b, :, :],
                        in0=p16[:, g + 4, :, :],
                        scalar=wsum_bc[:, b, g + 4:g + 5],
                        in1=mh[:, b, :, :],
                        op0=mybir.AluOpType.mult,
                        op1=mybir.AluOpType.add,
                    )
                nc.vector.tensor_tensor(
                    out=m16[:, b, :, :],
                    in0=m16[:, b, :, :],
                    in1=mh[:, b, :, :],
                    op=mybir.AluOpType.add,
                )

            # ---- K. out[b, s, fsl] = x[b] @ M[b] + cvec  (s on partitions) ----
            for b in range(B):
                for hf in range(2):
                    psum_o = ppo.tile([P, 8, FSL], f32, tag="po")
                    for jj in range(8):
                        j = hf * 8 + jj
                        for dc in range(DC):
                            nc.tensor.matmul(
                                psum_o[:, jj, :],
                                lhsT=x_sb[:, dc, b, j * P:(j + 1) * P],
                                rhs=m16[:, b, dc, :],
                                start=(dc == 0),
                                stop=(dc == DC - 1),
                            )
                    if hf == 0:
                        nc.scalar.activation(
                            out=out_sb[:, hf * 8:(hf + 1) * 8, b, :],
                            in_=psum_o[:, :, :],
                            func=mybir.ActivationFunctionType.Identity,
                        )
                    else:
                        nc.vector.tensor_copy(
                            out_sb[:, hf * 8:(hf + 1) * 8, b, :],
                            psum_o[:, :, :],
                        )
                    nc.sync.dma_start(
                        out=out_d[b, hf * 8:(hf + 1) * 8, :, :].rearrange(
                            "j p f -> p j f"
                        ),
                        in_=out_sb[:, hf * 8:(hf + 1) * 8, b, :],
                    )

            # ---- J. cvec[b] = sum_g wsum[b,g]*bvo[:,g] + bo; flip to [1,(b f)] ----
            for b in range(B):
                nc.vector.scalar_tensor_tensor(
                    out=cvec_sb[:, b:b + 1],
                    in0=bvo_sb[:, 0:1],
                    scalar=wsum_bc[0:FSL, b, 0:1],
                    in1=bo_sb[:, :],
                    op0=mybir.AluOpType.mult,
                    op1=mybir.AluOpType.add,
                )
                for g in range(1, G):
                    nc.vector.scalar_tensor_tensor(
                        out=cvec_sb[:, b:b + 1],
                        in0=bvo_sb[:, g:g + 1],
                        scalar=wsum_bc[0:FSL, b, g:g + 1],
                        in1=cvec_sb[:, b:b + 1],
                        op0=mybir.AluOpType.mult,
                        op1=mybir.AluOpType.add,
                    )
            nc.sync.dma_start(out=cv_d[:, :], in_=cvec_sb[:, :])


    nc.compile()
    return nc


def kernel(x, Wq, bq, Wk, bk, Wv, bv, Wo, bo):
    from concourse.bass_utils import run_bass_kernel_spmd

    if "nc" not in _cache:
        _cache["nc"] = _build_nc()
    nc = _cache["nc"]

    f16 = np.float16
    xT16 = np.ascontiguousarray(
        np.asarray(x, np.float32).transpose(2, 0, 1)).astype(f16)  # [d,b,s]
    wq_r = np.asarray(Wq, np.float32).reshape(D, G, H, D)
    wvT16 = np.ascontiguousarray(
        np.asarray(Wv, np.float32).reshape(D, G, D).transpose(1, 2, 0)
    ).astype(f16)                                                   # [g,e,d]
    wo_r = np.asarray(Wo, np.float32).reshape(G, D, D)
    bq_r = np.asarray(bq, np.float32).reshape(G, H, D)
    in_maps = []
    for c in range(N_CORES):
        fs = slice(c * FSL, (c + 1) * FSL)
        in_maps.append({
            "xT16": xT16,
            "wk16": np.ascontiguousarray(
                np.asarray(Wk, np.float32)[:, c * D:(c + 1) * D]).astype(f16),
            "wq16": np.ascontiguousarray(
                wq_r[:, c].transpose(2, 1, 0)).astype(
                    __import__("ml_dtypes").float8_e4m3),            # [a,h,e]
            "wvT16": wvT16,
            "wo16": np.ascontiguousarray(
                wo_r[:, :, fs].transpose(1, 0, 2)).astype(f16),      # [e,g,f]
            "bq16": np.ascontiguousarray(bq_r[c].T).astype(f16),     # [k,h]
            "bk32": np.ascontiguousarray(
                np.asarray(bk, np.float32)[c * D:(c + 1) * D]),
            "bv32": np.ascontiguousarray(
                np.asarray(bv, np.float32).reshape(G, D).T),         # [e,g]
            "bo32": np.ascontiguousarray(np.asarray(bo, np.float32)[fs]),
        })
    res = run_bass_kernel_spmd(nc, in_maps, core_ids=list(range(N_CORES)))
    _cache["last_results"] = res
    full = np.concatenate(
        [r["out16"].reshape(B, S, FSL) for r in res.results], axis=2
    ).astype(np.float32)                              # [B, S, D]
    cvec = np.concatenate(
        [r["cvec32"].T for r in res.results], axis=1
    )                                                 # [B, D]
    return full + cvec[:, None, :]


# revision 54
# speedup vs baseline: 1.0204x; 1.0043x over previous
"""Trainium2 Bass kernel for nn_GroupedQueryAttention_86380382257377.

Math: the reference einsums collapse —
  scores[b,q,h,g] = x[b,q,:] . wq_eff[b][:, g, h] + bqdot[b,g,h]
      with wq_eff[b][e,(g,h)] = sum_k Wq[e,(g,h),k] * ksum[b,g,k],
           ksum[b,g] = Wk_g^T xs[b] + S*bk_g,  xs[b] = sum_s x[b,s,:]
  weights = softmax_g(scores);  wsum[b,g] = sum_{q,h} weights
  out[b]  = x[b] @ M[b] + cvec[b],
      M[b] = sum_g wsum[b,g] * (Wv_g @ Wo_g),
      cvec[b] = sum_g wsum[b,g] * (bv_g @ Wo_g) + bo.

Sharding (8 cores): core c owns group c for the Wq/Wk shard (one small
fp16 AllGather of (wq_eff, bqdot)); x is replicated (fp16); the P = Wv@Wo
and x@M stages are column-sharded (64 output cols per core).  All heavy
matmuls and DMAs run in fp16 (PSUM accumulation stays fp32).
"""

import numpy as np

B, S, D, G, H = 2, 2048, 512, 8, 4
N_CORES = 8
FSL = D // N_CORES  # 64 output columns per core
P = 128
DC = D // P   # 4 chunks of the contraction dims
JC = S // P   # 16 score row-chunks
SC = S // 512  # 4 out column-chunks
NXCH = 4      # x DMA chunks (for overlapped xs reduction)
XCW = S // NXCH  # 512
INV_SQRT_D = 1.0 / float(np.sqrt(D))
CHUNK = D * B * H + B * H  # 4096 wq_eff + 8 bqdot  (fp16 elements)

_cache = {}


def _build_nc():
    import concourse.bass as bass
    import concourse.mybir as mybir
    import concourse.tile as tile
    from concourse import bacc

    f32 = mybir.dt.float32
    f16 = mybir.dt.float16
    f8 = mybir.dt.float8e4
    nc = bacc.Bacc(None, num_devices=N_CORES)

    # ---- kernel I/O (host-prepared, fp16 unless noted) ----
    xT_d = nc.dram_tensor("xT16", [D, B, S], f16, kind="ExternalInput")     # [d,b,s]
    wk_d = nc.dram_tensor("wk16", [D, D], f16, kind="ExternalInput")        # [d,k]
    wq_d = nc.dram_tensor("wq16", [D, H, D], f8, kind="ExternalInput")      # [a,h,e]
    wvT_d = nc.dram_tensor("wvT16", [G, D, D], f16, kind="ExternalInput")   # [g,e,d]
    wo_d = nc.dram_tensor("wo16", [D, G, FSL], f16, kind="ExternalInput")   # [e,g,f]
    bq_d = nc.dram_tensor("bq16", [D, H], f16, kind="ExternalInput")        # [k,h]
    bk_d = nc.dram_tensor("bk32", [D], f32, kind="ExternalInput")
    bv_d = nc.dram_tensor("bv32", [D, G], f32, kind="ExternalInput")        # [e,g]
    bo_d = nc.dram_tensor("bo32", [FSL], f32, kind="ExternalInput")
    out_d = nc.dram_tensor("out16", [B, JC, P, FSL], f16, kind="ExternalOutput")
    cv_d = nc.dram_tensor("cvec32", [FSL, B], f32, kind="ExternalOutput")

    with tile.TileContext(nc) as tc:
        with (
            tc.tile_pool(name="sing", bufs=1) as sing,
            tc.tile_pool(name="pps", bufs=1, space="PSUM") as pps,
            tc.tile_pool(name="pp", bufs=2, space="PSUM") as pp,
            tc.tile_pool(name="pss", bufs=2, space="PSUM") as pss,
            tc.tile_pool(name="pws", bufs=1, space="PSUM") as pws,
            tc.tile_pool(name="ppo", bufs=2, space="PSUM") as ppo,
            tc.tile_pool(name="dram", bufs=1, space="DRAM") as dram,
        ):
            # ---- persistent SBUF tiles ----
            x_sb = sing.tile([P, DC, B, S], f16)          # 32KB/part
            red = sing.tile([P, DC, B, 1024], f16)        # xs tree scratch
            wv_sb = sing.tile([P, G, DC, D], f16)         # lhsT [e, d] per (g,ec)
            wq_sb = sing.tile([P, DC, H, D], f8)          # lhsT [a, e] per (h,ac)
            wk_sb = sing.tile([P, DC, D], f16)            # lhsT [d, k] per (kc,dc)
            wo_sb = sing.tile([P, DC, G, FSL], f16)       # rhs [e, f] per (g,ec)
            bq_sb = sing.tile([P, DC, H], f16)            # rhs [k, h]
            bk_sb = sing.tile([P, DC], f32)
            bkS_sb = sing.tile([P, DC], f32)
            bv16 = sing.tile([P, DC, G], f16)
            bvo_sb = sing.tile([FSL, G], f32)
            bo_sb = sing.tile([FSL, 1], f32)
            cvec_sb = sing.tile([FSL, B], f32)
            ones_sb = sing.tile([P, P], f16)
            xs32 = sing.tile([P, DC, B], f32)
            xs16 = sing.tile([P, DC, B], f16)
            ksum16 = sing.tile([P, DC, B], f16)
            wqe_loc = sing.tile([P, DC, B, H], f8)
            bqd_loc = sing.tile([B, H], f8)
            wqe_all = sing.tile([P, G, DC, B, H], f8)
            bqd_all = sing.tile([1, B, G, H], f8)
            s1_sb = sing.tile([P, B, JC, G, H], f32)      # exp(scores)
            den_sb = sing.tile([P, B, JC, H], f32)
            rec_sb = sing.tile([P, B, JC, H], f32)
            w16_sb = sing.tile([P, B, JC, G, H], f16)     # softmax weights
            wsum_sb = sing.tile([1, B, G], f32)
            ws16_sb = sing.tile([1, B * G], f16)
            wsum_bc = sing.tile([P, B, G], f32)
            p16 = sing.tile([P, G, DC, FSL], f16)         # P_g[:, fslice]
            m16 = sing.tile([P, B, DC, FSL], f16)         # M[b][:, fslice]
            out_sb = sing.tile([P, JC, B, FSL], f16)

            # ---- internal DRAM (collective bounce) ----
            wq_bounce = dram.tile([CHUNK], f8)
            wq_gath = dram.tile([G * CHUNK], f8)

            nc.vector.memset(ones_sb[:, :], 1.0)

            # ---- input DMAs, ordered for the critical path:
            #      x chunks (xs tree), wk, wq  ->  AllGather chain
            #      wv, wo, biases              ->  P / cvec path
            nc.sync.dma_start(
                out=wq_sb[:, :, :, :], in_=wq_d.rearrange("(ac p) h e -> p ac h e", p=P)
            )
            for dc in range(DC):
                for hh in range(2):
                    nc.sync.dma_start(
                        out=x_sb[:, dc, :, hh * 1024:(hh + 1) * 1024],
                        in_=xT_d[dc * P:(dc + 1) * P, :, hh * 1024:(hh + 1) * 1024],
                    )
            nc.sync.dma_start(
                out=wk_sb[:, :, :], in_=wk_d.rearrange("(dc p) k -> p dc k", p=P)
            )
            nc.sync.dma_start(
                out=bk_sb[:, :], in_=bk_d.rearrange("(dc p) -> p dc", p=P)
            )
            nc.sync.dma_start(
                out=bq_sb[:, :, :], in_=bq_d.rearrange("(kc p) h -> p kc h", p=P)
            )

            # ---- A. xs[b,d] = sum_s x : fp16 halving tree per (dc, s-half) ----
            for dc in range(DC):
                for hh in range(2):
                    hb = hh * 1024
                    rb = hh * 512
                    nc.vector.tensor_tensor(
                        out=red[:, dc, :, rb:rb + 512],
                        in0=x_sb[:, dc, :, hb:hb + 512],
                        in1=x_sb[:, dc, :, hb + 512:hb + 1024],
                        op=mybir.AluOpType.add,
                    )
                    w = 256
                    while w >= 8:
                        nc.vector.tensor_tensor(
                            out=red[:, dc, :, rb:rb + w],
                            in0=red[:, dc, :, rb:rb + w],
                            in1=red[:, dc, :, rb + w:rb + 2 * w],
                            op=mybir.AluOpType.add,
                        )
                        w //= 2
            nc.vector.tensor_reduce(
                out=xs32[:, :, :],
                in_=red[:, :, :, :].rearrange(
                    "p dc b (hh o) -> p dc b hh o", hh=2
                )[:, :, :, :, 0:8],
                axis=mybir.AxisListType.XY,
                op=mybir.AluOpType.add,
            )
            nc.vector.tensor_copy(xs16[:, :, :], xs32[:, :, :])

            # ---- B. ksumT[k,b] = Wk_c^T xs + S*bk ----
            nc.vector.tensor_scalar_mul(bkS_sb[:, :], bk_sb[:, :], float(S))
            psmall = pps.tile([P, 512], f32, tag="small")
            psum_k = psmall[:, 0:8].rearrange("p (kc b) -> p kc b", kc=DC)
            for kc in range(DC):
                for dc in range(DC):
                    nc.tensor.matmul(
                        psum_k[:, kc, :],
                        lhsT=wk_sb[:, dc, kc * P:(kc + 1) * P],
                        rhs=xs16[:, dc, :],
                        start=(dc == 0),
                        stop=(dc == DC - 1),
                    )
            bk_b = bkS_sb[:, :]
            nc.vector.tensor_tensor(
                out=ksum16[:, :, :],
                in0=psum_k[:, :, :],
                in1=bass.AP(
                    tensor=bk_b.tensor, offset=bk_b.offset,
                    ap=list(bk_b.ap) + [[0, B]],
                ),
                op=mybir.AluOpType.add,
            )

            # ---- C. wq_eff[e,(b)] per (h, ec); bqdot[b,h]; scale; bounce ----
            psum_wq = psmall[:, 8:40].rearrange(
                "p (ec b h) -> p ec b h", ec=DC, b=B
            )
            for h in range(H):
                for ec in range(DC):
                    for kc in range(DC):
                        nc.tensor.matmul(
                            psum_wq[:, ec, :, h],
                            lhsT=wq_sb[:, kc, h, ec * P:(ec + 1) * P],
                            rhs=ksum16[:, kc, :],
                            start=(kc == 0),
                            stop=(kc == DC - 1),
                        )
            psum_bqd = psmall[0:B, 40:44]
            for kc in range(DC):
                nc.tensor.matmul(
                    psum_bqd[:, :],
                    lhsT=ksum16[:, kc, :],
                    rhs=bq_sb[:, kc, :],
                    start=(kc == 0),
                    stop=(kc == DC - 1),
                )
            nc.vector.tensor_scalar_mul(wqe_loc[:, :, :, :], psum_wq[:, :, :, :], INV_SQRT_D)
            nc.vector.tensor_scalar_mul(bqd_loc[:, :], psum_bqd[:, :], INV_SQRT_D)
            nc.sync.dma_start(
                out=wq_bounce[0:D * B * H].rearrange(
                    "(p ac b h) -> p ac b h", p=P, ac=DC, b=B
                ),
                in_=wqe_loc[:, :, :, :],
            )
            nc.sync.dma_start(
                out=wq_bounce[D * B * H:CHUNK].rearrange("(b h) -> b h", b=B),
                in_=bqd_loc[:, :],
            )

            # ---- D2. weight DMAs for the P path (the AllGather bounce slots
            #      between the 1MB chunks) ----
            for g in range(G):
                nc.sync.dma_start(
                    out=wv_sb[:, g:g + 1, :, :],
                    in_=wvT_d[g:g + 1, :, :].rearrange(
                        "g (ec p) d -> p g ec d", p=P
                    ),
                )
            for eh in range(2):
                nc.sync.dma_start(
                    out=wo_sb[:, 2 * eh:2 * eh + 2, :, :],
                    in_=wo_d[eh * 256:(eh + 1) * 256, :, :].rearrange(
                        "(ec p) g f -> p ec g f", p=P
                    ),
                )
            nc.vector.tensor_copy(bv16[0:1, 0, 0:4], wk_sb[0:1, 0, 0:4])
            nc.vector.tensor_copy(bo_sb[0:1, 0:1], wk_sb[0:1, 0, 0:1])
            nc.gpsimd.dma_start(
                out=bv16[:, :, :], in_=bv_d.rearrange("(ec p) g -> p ec g", p=P)
            )
            nc.sync.dma_start(
                out=bo_sb[:, :], in_=bo_d.rearrange("(f o) -> f o", o=1)
            )

            # ---- D. AllGather of (wq_eff, bqdot), fp16 ----
            nc.gpsimd.collective_compute(
                "AllGather",
                mybir.AluOpType.bypass,
                replica_groups=[list(range(N_CORES))],
                ins=[wq_bounce[:].opt()],
                outs=[wq_gath[:].opt()],
            )

            # ---- E. spread gathered results ----
            gap = wq_gath[:]
            nc.sync.dma_start(
                out=wqe_all[:, :, :, :, :],
                in_=bass.AP(
                    tensor=gap.tensor,
                    offset=gap.offset,
                    ap=[[DC * B * H, P], [CHUNK, G], [1, DC * B * H]],
                ),
            )
            nc.sync.dma_start(
                out=bqd_all[:, :, :, :],
                in_=bass.AP(
                    tensor=gap.tensor,
                    offset=gap.offset + D * B * H,
                    ap=[[0, 1], [H, B], [CHUNK, G], [1, H]],
                ),
            )

            # ---- F. P_g = Wv_g @ Wo_g[:, fsl]  (all groups, f-slice) ----
            for g in range(G):
                psum_p = pp.tile([P, DC, FSL], f32, tag="pp")
                for dc in range(DC):
                    for ec in range(DC):
                        nc.tensor.matmul(
                            psum_p[:, dc, :],
                            lhsT=wv_sb[:, g, ec, dc * P:(dc + 1) * P],
                            rhs=wo_sb[:, ec, g, :],
                            start=(ec == 0),
                            stop=(ec == DC - 1),
                        )
                nc.scalar.activation(
                    out=p16[:, g, :, :],
                    in_=psum_p[:, :, :],
                    func=mybir.ActivationFunctionType.Copy,
                )

            # ---- F2. bvo[f, g] = bv_g @ Wo_g[:, fsl]  (early) ----
            psum_bvo = psmall[0:FSL, 192:200]
            for g in range(G):
                for ec in range(DC):
                    nc.tensor.matmul(
                        psum_bvo[:, g:g + 1],
                        lhsT=wo_sb[:, ec, g, :],
                        rhs=bv16[:, ec, g:g + 1],
                        start=(ec == 0),
                        stop=(ec == DC - 1),
                    )
            nc.vector.tensor_copy(bvo_sb[:, :], psum_bvo[:, :])

            # ---- G. scores + exp + softmax + wsum (full sequence) ----
            for b in range(B):
                psum_s = pss.tile([P, JC, G * H], f32, tag="ps")
                bq_b = bqd_all[:, b, :, :]
                nc.tensor.matmul(
                    psum_s[:, :, :],
                    lhsT=ones_sb[0:1, :],
                    rhs=bass.AP(
                        tensor=bq_b.tensor, offset=bq_b.offset,
                        ap=[list(bq_b.ap[0]), [0, JC]] + list(bq_b.ap[1:]),
                    ),
                    start=True,
                    stop=False,
                )
                for j in range(JC):
                    for dc in range(DC):
                        nc.tensor.matmul(
                            psum_s[:, j, :],
                            lhsT=x_sb[:, dc, b, j * P:(j + 1) * P],
                            rhs=wqe_all[:, :, dc, b, :],
                            start=False,
                            stop=(j == JC - 1 and dc == DC - 1),
                            skip_group_check=True,
                        )
                nc.scalar.activation(
                    out=s1_sb[:, b, :, :, :].rearrange("p j g h -> p j (g h)"),
                    in_=psum_s[:, :, :],
                    func=mybir.ActivationFunctionType.Exp,
                )
                nc.vector.tensor_reduce(
                    out=den_sb[:, b, :, :],
                    in_=s1_sb[:, b, :, :, :].rearrange("p j g h -> p j h g"),
                    axis=mybir.AxisListType.X,
                    op=mybir.AluOpType.add,
                )
                nc.vector.reciprocal(rec_sb[:, b, :, :], den_sb[:, b, :, :])
                rb = rec_sb[:, b, :, :]
                nc.vector.tensor_tensor(
                    out=w16_sb[:, b, :, :, :].rearrange("p j g h -> p j h g"),
                    in0=s1_sb[:, b, :, :, :].rearrange("p j g h -> p j h g"),
                    in1=bass.AP(
                        tensor=rb.tensor,
                        offset=rb.offset,
                        ap=list(rb.ap) + [[0, G]],
                    ),
                    op=mybir.AluOpType.mult,
                )
                psum_ws = pws.tile([P, JC * G * H], f32, tag="ws")
                nc.tensor.matmul(
                    psum_ws[:, :],
                    lhsT=ones_sb[:, :],
                    rhs=w16_sb[:, b, :, :, :],
                    start=True,
                    stop=True,
                )
                nc.vector.tensor_reduce(
                    out=wsum_bc[:, b, :],
                    in_=psum_ws[:, :].rearrange("p (j g h) -> p g j h", j=JC, g=G),
                    axis=mybir.AxisListType.XY,
                    op=mybir.AluOpType.add,
                )

            # ---- H2. PE warm-up fillers: keep the tensor engine busy through
            #      the softmax/combine window so the out matmuls run at full
            #      clock (cheap redundant column-sums into a recycled bank) ----
            for _ in range(32):
                psum_fill = pss.tile([P, JC, G * H], f32, tag="ps")
                nc.tensor.matmul(
                    psum_fill[:, :, :],
                    lhsT=ones_sb[:, :],
                    rhs=w16_sb[:, 0, :, :, :],
                    start=True,
                    stop=True,
                )

            # ---- I. M[b] = sum_g wsum[b,g] * P_g ----
            mh = sing.tile([P, B, DC, FSL], f16)
            for b in range(B):
                nc.vector.tensor_scalar(
                    out=m16[:, b, :, :],
                    in0=p16[:, 0, :, :],
                    scalar1=wsum_bc[:, b, 0:1],
                    scalar2=None,
                    op0=mybir.AluOpType.mult,
                )
                nc.vector.tensor_scalar(
                    out=mh[:, b, :, :],
                    in0=p16[:, 4, :, :],
                    scalar1=wsum_bc[:, b, 4:5],
                    scalar2=None,
                    op0=mybir.AluOpType.mult,
                )
                for g in (1, 2, 3):
                    nc.vector.scalar_tensor_tensor(
                        out=m16[:, b, :, :],
                        in0=p16[:, g, :, :],
                        scalar=wsum_bc[:, b, g:g + 1],
                        in1=m16[:, b, :, :],
                        op0=mybir.AluOpType.mult,
                        op1=mybir.AluOpType.add,
                    )
                    nc.vector.scalar_tensor_tensor(
                        out=mh[:, b, :, :],
                        in0=p16[:, g + 4, :, :],
                        scalar=wsum_bc[:, b, g + 4:g + 5],
                        in1=mh[:, b, :, :],
                        op0=mybir.AluOpType.mult,
                        op1=mybir.AluOpType.add,
                    )
                nc.vector.tensor_tensor(
                    out=m16[:, b, :, :],
                    in0=m16[:, b, :, :],
                    in1=mh[:, b, :, :],
                    op=mybir.AluOpType.add,
                )

            # ---- K. out[b, s, fsl] = x[b] @ M[b] + cvec  (s on partitions) ----
            for b in range(B):
                for hf in range(2):
                    psum_o = ppo.tile([P, 8, FSL], f32, tag="po")
                    for jj in range(8):
                        j = hf * 8 + jj
                        for dc in range(DC):
                            nc.tensor.matmul(
                                psum_o[:, jj, :],
                                lhsT=x_sb[:, dc, b, j * P:(j + 1) * P],
                                rhs=m16[:, b, dc, :],
                                start=(dc == 0),
                                stop=(dc == DC - 1),
                            )
                    if hf == 0:
                        nc.scalar.activation(
                            out=out_sb[:, hf * 8:(hf + 1) * 8, b, :],
                            in_=psum_o[:, :, :],
                            func=mybir.ActivationFunctionType.Identity,
                        )
                    else:
                        nc.vector.tensor_copy(
                            out_sb[:, hf * 8:(hf + 1) * 8, b, :],
                            psum_o[:, :, :],
                        )
                    nc.sync.dma_start(
                        out=out_d[b, hf * 8:(hf + 1) * 8, :, :].rearrange(
                            "j p f -> p j f"
                        ),
                        in_=out_sb[:, hf * 8:(hf + 1) * 8, b, :],
                    )

            # ---- J. cvec[b] = sum_g wsum[b,g]*bvo[:,g] + bo; flip to [1,(b f)] ----
            for b in range(B):
                nc.vector.scalar_tensor_tensor(
                    out=cvec_sb[:, b:b + 1],
                    in0=bvo_sb[:, 0:1],
                    scalar=wsum_bc[0:FSL, b, 0:1],
                    in1=bo_sb[:, :],
                    op0=mybir.AluOpType.mult,
                    op1=mybir.AluOpType.add,
                )
                for g in range(1, G):
                    nc.vector.scalar_tensor_tensor(
                        out=cvec_sb[:, b:b + 1],
                        in0=bvo_sb[:, g:g + 1],
                        scalar=wsum_bc[0:FSL, b, g:g + 1],
                        in1=cvec_sb[:, b:b + 1],
                        op0=mybir.AluOpType.mult,
                        op1=mybir.AluOpType.add,
                    )
            nc.sync.dma_start(out=cv_d[:, :], in_=cvec_sb[:, :])


    nc.compile()
    return nc


def kernel(x, Wq, bq, Wk, bk, Wv, bv, Wo, bo):
    from concourse.bass_utils import run_bass_kernel_spmd

    if "nc" not in _cache:
        _cache["nc"] = _build_nc()
    nc = _cache["nc"]

    f16 = np.float16
    xT16 = np.ascontiguousarray(
        np.asarray(x, np.float32).transpose(2, 0, 1)).astype(f16)  # [d,b,s]
    wq_r = np.asarray(Wq, np.float32).reshape(D, G, H, D)
    wvT16 = np.ascontiguousarray(
        np.asarray(Wv, np.float32).reshape(D, G, D).transpose(1, 2, 0)
    ).astype(f16)                                                   # [g,e,d]
    wo_r = np.asarray(Wo, np.float32).reshape(G, D, D)
    bq_r = np.asarray(bq, np.float32).reshape(G, H, D)
    in_maps = []
    for c in range(N_CORES):
        fs = slice(c * FSL, (c + 1) * FSL)
        in_maps.append({
            "xT16": xT16,
            "wk16": np.ascontiguousarray(
                np.asarray(Wk, np.float32)[:, c * D:(c + 1) * D]).astype(f16),
            "wq16": np.ascontiguousarray(
                wq_r[:, c].transpose(2, 1, 0)).astype(
                    __import__("ml_dtypes").float8_e4m3),            # [a,h,e]
            "wvT16": wvT16,
            "wo16": np.ascontiguousarray(
                wo_r[:, :, fs].transpose(1, 0, 2)).astype(f16),      # [e,g,f]
            "bq16": np.ascontiguousarray(bq_r[c].T).astype(f16),     # [k,h]
            "bk32": np.ascontiguousarray(
                np.asarray(bk, np.float32)[c * D:(c + 1) * D]),
            "bv32": np.ascontiguousarray(
                np.asarray(bv, np.float32).reshape(G, D).T),         # [e,g]
            "bo32": np.ascontiguousarray(np.asarray(bo, np.float32)[fs]),
        })
    res = run_bass_kernel_spmd(nc, in_maps, core_ids=list(range(N_CORES)))
    _cache["last_results"] = res
    full = np.concatenate(
        [r["out16"].reshape(B, S, FSL) for r in res.results], axis=2
    ).astype(np.float32)                              # [B, S, D]
    cvec = np.concatenate(
        [r["cvec32"].T for r in res.results], axis=1
    )                                                 # [B, D]
    return full + cvec[:, None, :]


# revision 57
# speedup vs baseline: 1.0247x; 1.0042x over previous
"""Trainium2 Bass kernel for nn_GroupedQueryAttention_86380382257377.

Math: the reference einsums collapse —
  scores[b,q,h,g] = x[b,q,:] . wq_eff[b][:, g, h] + bqdot[b,g,h]
      with wq_eff[b][e,(g,h)] = sum_k Wq[e,(g,h),k] * ksum[b,g,k],
           ksum[b,g] = Wk_g^T xs[b] + S*bk_g,  xs[b] = sum_s x[b,s,:]
  weights = softmax_g(scores);  wsum[b,g] = sum_{q,h} weights
  out[b]  = x[b] @ M[b] + cvec[b],
      M[b] = sum_g wsum[b,g] * (Wv_g @ Wo_g),
      cvec[b] = sum_g wsum[b,g] * (bv_g @ Wo_g) + bo.

Sharding (8 cores): core c owns group c for the Wq/Wk shard (one small
fp16 AllGather of (wq_eff, bqdot)); x is replicated (fp16); the P = Wv@Wo
and x@M stages are column-sharded (64 output cols per core).  All heavy
matmuls and DMAs run in fp16 (PSUM accumulation stays fp32).
"""

import numpy as np

B, S, D, G, H = 2, 2048, 512, 8, 4
N_CORES = 8
FSL = D // N_CORES  # 64 output columns per core
P = 128
DC = D // P   # 4 chunks of the contraction dims
JC = S // P   # 16 score row-chunks
SC = S // 512  # 4 out column-chunks
NXCH = 4      # x DMA chunks (for overlapped xs reduction)
XCW = S // NXCH  # 512
INV_SQRT_D = 1.0 / float(np.sqrt(D))
WQC = 36                   # 32 wq_eff cols + 4 bqdot cols (padded)
CHUNK = P * WQC            # fp8 elements per rank in the AllGather

_cache = {}


def _build_nc():
    import concourse.bass as bass
    import concourse.mybir as mybir
    import concourse.tile as tile
    from concourse import bacc

    f32 = mybir.dt.float32
    f16 = mybir.dt.float16
    f8 = mybir.dt.float8e4
    nc = bacc.Bacc(None, num_devices=N_CORES)

    # ---- kernel I/O (host-prepared, fp16 unless noted) ----
    xT_d = nc.dram_tensor("xT16", [D, B, S], f16, kind="ExternalInput")     # [d,b,s]
    wk_d = nc.dram_tensor("wk16", [D, D], f16, kind="ExternalInput")        # [d,k]
    wq_d = nc.dram_tensor("wq16", [D, H, D], f8, kind="ExternalInput")      # [a,h,e]
    wvT_d = nc.dram_tensor("wvT16", [G, D, D], f16, kind="ExternalInput")   # [g,e,d]
    wo_d = nc.dram_tensor("wo16", [D, G, FSL], f16, kind="ExternalInput")   # [e,g,f]
    bq_d = nc.dram_tensor("bq16", [D, H], f16, kind="ExternalInput")        # [k,h]
    bk_d = nc.dram_tensor("bk32", [D], f32, kind="ExternalInput")
    bv_d = nc.dram_tensor("bv32", [D, G], f32, kind="ExternalInput")        # [e,g]
    bo_d = nc.dram_tensor("bo32", [FSL], f32, kind="ExternalInput")
    out_d = nc.dram_tensor("out16", [B, JC, P, FSL], f16, kind="ExternalOutput")
    cv_d = nc.dram_tensor("cvec32", [FSL, B], f32, kind="ExternalOutput")

    with tile.TileContext(nc) as tc:
        with (
            tc.tile_pool(name="sing", bufs=1) as sing,
            tc.tile_pool(name="pps", bufs=1, space="PSUM") as pps,
            tc.tile_pool(name="pp", bufs=2, space="PSUM") as pp,
            tc.tile_pool(name="pss", bufs=2, space="PSUM") as pss,
            tc.tile_pool(name="pws", bufs=1, space="PSUM") as pws,
            tc.tile_pool(name="ppo", bufs=2, space="PSUM") as ppo,
            tc.tile_pool(name="dram", bufs=1, space="DRAM") as dram,
        ):
            # ---- persistent SBUF tiles ----
            x_sb = sing.tile([P, DC, B, S], f16)          # 32KB/part
            red = sing.tile([P, DC, B, 1024], f16)        # xs tree scratch
            wv_sb = sing.tile([P, G, DC, D], f16)         # lhsT [e, d] per (g,ec)
            wq_sb = sing.tile([P, DC, H, D], f8)          # lhsT [a, e] per (h,ac)
            wk_sb = sing.tile([P, DC, D], f16)            # lhsT [d, k] per (kc,dc)
            wo_sb = sing.tile([P, DC, G, FSL], f16)       # rhs [e, f] per (g,ec)
            bq_sb = sing.tile([P, DC, H], f16)            # rhs [k, h]
            bk_sb = sing.tile([P, DC], f32)
            bkS_sb = sing.tile([P, DC], f32)
            bv16 = sing.tile([P, DC, G], f16)
            bvo_sb = sing.tile([FSL, G], f32)
            bo_sb = sing.tile([FSL, 1], f32)
            cvec_sb = sing.tile([FSL, B], f32)
            ones_sb = sing.tile([P, P], f16)
            xs32 = sing.tile([P, DC, B], f32)
            xs16 = sing.tile([P, DC, B], f16)
            ksum16 = sing.tile([P, DC, B], f16)
            wqe2 = sing.tile([P, WQC], f8)
            wqe_all = sing.tile([P, G, DC, B, H], f8)
            bqd_all = sing.tile([1, B, G, H], f8)
            s1_sb = sing.tile([P, B, JC, G, H], f32)      # exp(scores)
            den_sb = sing.tile([P, B, JC, H], f32)
            rec_sb = sing.tile([P, B, JC, H], f32)
            w16_sb = sing.tile([P, B, JC, G, H], f16)     # softmax weights
            wsum_sb = sing.tile([1, B, G], f32)
            ws16_sb = sing.tile([1, B * G], f16)
            wsum_bc = sing.tile([P, B, G], f32)
            p16 = sing.tile([P, G, DC, FSL], f16)         # P_g[:, fslice]
            m16 = sing.tile([P, B, DC, FSL], f16)         # M[b][:, fslice]
            out_sb = sing.tile([P, JC, B, FSL], f16)

            # ---- internal DRAM (collective bounce) ----
            wq_bounce = dram.tile([CHUNK], f8)
            wq_gath = dram.tile([G * CHUNK], f8)

            nc.vector.memset(ones_sb[:, :], 1.0)
            nc.vector.memset(wqe2[:, 32:WQC], 0.0)

            # ---- input DMAs, ordered for the critical path:
            #      x chunks (xs tree), wk, wq  ->  AllGather chain
            #      wv, wo, biases              ->  P / cvec path
            nc.sync.dma_start(
                out=wq_sb[:, :, :, :], in_=wq_d.rearrange("(ac p) h e -> p ac h e", p=P)
            )
            for dc in range(DC):
                for hh in range(2):
                    nc.sync.dma_start(
                        out=x_sb[:, dc, :, hh * 1024:(hh + 1) * 1024],
                        in_=xT_d[dc * P:(dc + 1) * P, :, hh * 1024:(hh + 1) * 1024],
                    )
            nc.sync.dma_start(
                out=wk_sb[:, :, :], in_=wk_d.rearrange("(dc p) k -> p dc k", p=P)
            )
            nc.sync.dma_start(
                out=bk_sb[:, :], in_=bk_d.rearrange("(dc p) -> p dc", p=P)
            )
            nc.sync.dma_start(
                out=bq_sb[:, :, :], in_=bq_d.rearrange("(kc p) h -> p kc h", p=P)
            )

            # ---- A. xs[b,d] = sum_s x : fp16 halving tree per (dc, s-half) ----
            for dc in range(DC):
                for hh in range(2):
                    hb = hh * 1024
                    rb = hh * 512
                    nc.vector.tensor_tensor(
                        out=red[:, dc, :, rb:rb + 512],
                        in0=x_sb[:, dc, :, hb:hb + 512],
                        in1=x_sb[:, dc, :, hb + 512:hb + 1024],
                        op=mybir.AluOpType.add,
                    )
                    w = 256
                    while w >= 8:
                        nc.vector.tensor_tensor(
                            out=red[:, dc, :, rb:rb + w],
                            in0=red[:, dc, :, rb:rb + w],
                            in1=red[:, dc, :, rb + w:rb + 2 * w],
                            op=mybir.AluOpType.add,
                        )
                        w //= 2
            nc.vector.tensor_reduce(
                out=xs32[:, :, :],
                in_=red[:, :, :, :].rearrange(
                    "p dc b (hh o) -> p dc b hh o", hh=2
                )[:, :, :, :, 0:8],
                axis=mybir.AxisListType.XY,
                op=mybir.AluOpType.add,
            )
            nc.vector.tensor_copy(xs16[:, :, :], xs32[:, :, :])

            # ---- B. ksumT[k,b] = Wk_c^T xs + S*bk ----
            nc.vector.tensor_scalar_mul(bkS_sb[:, :], bk_sb[:, :], float(S))
            psmall = pps.tile([P, 512], f32, tag="small")
            psum_k = psmall[:, 0:8].rearrange("p (kc b) -> p kc b", kc=DC)
            for kc in range(DC):
                for dc in range(DC):
                    nc.tensor.matmul(
                        psum_k[:, kc, :],
                        lhsT=wk_sb[:, dc, kc * P:(kc + 1) * P],
                        rhs=xs16[:, dc, :],
                        start=(dc == 0),
                        stop=(dc == DC - 1),
                    )
            bk_b = bkS_sb[:, :]
            nc.vector.tensor_tensor(
                out=ksum16[:, :, :],
                in0=psum_k[:, :, :],
                in1=bass.AP(
                    tensor=bk_b.tensor, offset=bk_b.offset,
                    ap=list(bk_b.ap) + [[0, B]],
                ),
                op=mybir.AluOpType.add,
            )

            # ---- C. wq_eff[e,(b)] per (h, ec); bqdot[b,h]; scale; bounce ----
            psum_wq = psmall[:, 8:40].rearrange(
                "p (ec b h) -> p ec b h", ec=DC, b=B
            )
            for h in range(H):
                for ec in range(DC):
                    for kc in range(DC):
                        nc.tensor.matmul(
                            psum_wq[:, ec, :, h],
                            lhsT=wq_sb[:, kc, h, ec * P:(ec + 1) * P],
                            rhs=ksum16[:, kc, :],
                            start=(kc == 0),
                            stop=(kc == DC - 1),
                        )
            psum_bqd = psmall[0:B, 40:44]
            for kc in range(DC):
                nc.tensor.matmul(
                    psum_bqd[:, :],
                    lhsT=ksum16[:, kc, :],
                    rhs=bq_sb[:, kc, :],
                    start=(kc == 0),
                    stop=(kc == DC - 1),
                )
            nc.vector.tensor_scalar_mul(
                wqe2[:, 0:32].rearrange("p (ac b h) -> p ac b h", ac=DC, b=B),
                psum_wq[:, :, :, :], INV_SQRT_D)
            nc.vector.tensor_scalar_mul(
                wqe2[0:B, 32:WQC], psum_bqd[:, :], INV_SQRT_D)
            nc.sync.dma_start(
                out=wq_bounce[:].rearrange("(p c) -> p c", p=P),
                in_=wqe2[:, :],
            )

            # ---- D2. weight DMAs for the P path (the AllGather bounce slots
            #      between the 1MB chunks) ----
            for g in range(G):
                nc.sync.dma_start(
                    out=wv_sb[:, g:g + 1, :, :],
                    in_=wvT_d[g:g + 1, :, :].rearrange(
                        "g (ec p) d -> p g ec d", p=P
                    ),
                )
            for eh in range(2):
                nc.sync.dma_start(
                    out=wo_sb[:, 2 * eh:2 * eh + 2, :, :],
                    in_=wo_d[eh * 256:(eh + 1) * 256, :, :].rearrange(
                        "(ec p) g f -> p ec g f", p=P
                    ),
                )
            nc.vector.tensor_copy(bv16[0:1, 0, 0:4], wk_sb[0:1, 0, 0:4])
            nc.vector.tensor_copy(bo_sb[0:1, 0:1], wk_sb[0:1, 0, 0:1])
            nc.gpsimd.dma_start(
                out=bv16[:, :, :], in_=bv_d.rearrange("(ec p) g -> p ec g", p=P)
            )
            nc.sync.dma_start(
                out=bo_sb[:, :], in_=bo_d.rearrange("(f o) -> f o", o=1)
            )

            # ---- D. AllGather of (wq_eff, bqdot), fp16 ----
            nc.gpsimd.collective_compute(
                "AllGather",
                mybir.AluOpType.bypass,
                replica_groups=[list(range(N_CORES))],
                ins=[wq_bounce[:].opt()],
                outs=[wq_gath[:].opt()],
            )

            # ---- E. spread gathered results ----
            gap = wq_gath[:]
            nc.sync.dma_start(
                out=wqe_all[:, :, :, :, :],
                in_=bass.AP(
                    tensor=gap.tensor,
                    offset=gap.offset,
                    ap=[[WQC, P], [CHUNK, G], [1, DC * B * H]],
                ),
            )
            nc.sync.dma_start(
                out=bqd_all[:, :, :, :],
                in_=bass.AP(
                    tensor=gap.tensor,
                    offset=gap.offset + 32,
                    ap=[[0, 1], [WQC, B], [CHUNK, G], [1, H]],
                ),
            )

            # ---- F. P_g = Wv_g @ Wo_g[:, fsl]  (all groups, f-slice) ----
            for g in range(G):
                psum_p = pp.tile([P, DC, FSL], f32, tag="pp")
                for dc in range(DC):
                    for ec in range(DC):
                        nc.tensor.matmul(
                            psum_p[:, dc, :],
                            lhsT=wv_sb[:, g, ec, dc * P:(dc + 1) * P],
                            rhs=wo_sb[:, ec, g, :],
                            start=(ec == 0),
                            stop=(ec == DC - 1),
                        )
                nc.scalar.activation(
                    out=p16[:, g, :, :],
                    in_=psum_p[:, :, :],
                    func=mybir.ActivationFunctionType.Copy,
                )

            # ---- F2. bvo[f, g] = bv_g @ Wo_g[:, fsl]  (early) ----
            psum_bvo = psmall[0:FSL, 192:200]
            for g in range(G):
                for ec in range(DC):
                    nc.tensor.matmul(
                        psum_bvo[:, g:g + 1],
                        lhsT=wo_sb[:, ec, g, :],
                        rhs=bv16[:, ec, g:g + 1],
                        start=(ec == 0),
                        stop=(ec == DC - 1),
                    )
            nc.vector.tensor_copy(bvo_sb[:, :], psum_bvo[:, :])

            # ---- G. scores + exp + softmax + wsum (full sequence) ----
            for b in range(B):
                psum_s = pss.tile([P, JC, G * H], f32, tag="ps")
                bq_b = bqd_all[:, b, :, :]
                nc.tensor.matmul(
                    psum_s[:, :, :],
                    lhsT=ones_sb[0:1, :],
                    rhs=bass.AP(
                        tensor=bq_b.tensor, offset=bq_b.offset,
                        ap=[list(bq_b.ap[0]), [0, JC]] + list(bq_b.ap[1:]),
                    ),
                    start=True,
                    stop=False,
                )
                for j in range(JC):
                    for dc in range(DC):
                        nc.tensor.matmul(
                            psum_s[:, j, :],
                            lhsT=x_sb[:, dc, b, j * P:(j + 1) * P],
                            rhs=wqe_all[:, :, dc, b, :],
                            start=False,
                            stop=(j == JC - 1 and dc == DC - 1),
                            skip_group_check=True,
                        )
                nc.scalar.activation(
                    out=s1_sb[:, b, :, :, :].rearrange("p j g h -> p j (g h)"),
                    in_=psum_s[:, :, :],
                    func=mybir.ActivationFunctionType.Exp,
                )
                nc.vector.tensor_reduce(
                    out=den_sb[:, b, :, :],
                    in_=s1_sb[:, b, :, :, :].rearrange("p j g h -> p j h g"),
                    axis=mybir.AxisListType.X,
                    op=mybir.AluOpType.add,
                )
                nc.vector.reciprocal(rec_sb[:, b, :, :], den_sb[:, b, :, :])
                rb = rec_sb[:, b, :, :]
                nc.vector.tensor_tensor(
                    out=w16_sb[:, b, :, :, :].rearrange("p j g h -> p j h g"),
                    in0=s1_sb[:, b, :, :, :].rearrange("p j g h -> p j h g"),
                    in1=bass.AP(
                        tensor=rb.tensor,
                        offset=rb.offset,
                        ap=list(rb.ap) + [[0, G]],
                    ),
                    op=mybir.AluOpType.mult,
                )
                psum_ws = pws.tile([P, JC * G * H], f32, tag="ws")
                nc.tensor.matmul(
                    psum_ws[:, :],
                    lhsT=ones_sb[:, :],
                    rhs=w16_sb[:, b, :, :, :],
                    start=True,
                    stop=True,
                )
                nc.vector.tensor_reduce(
                    out=wsum_bc[:, b, :],
                    in_=psum_ws[:, :].rearrange("p (j g h) -> p g j h", j=JC, g=G),
                    axis=mybir.AxisListType.XY,
                    op=mybir.AluOpType.add,
                )

            # ---- H2. PE warm-up fillers: keep the tensor engine busy through
            #      the softmax/combine window so the out matmuls run at full
            #      clock (cheap redundant column-sums into a recycled bank) ----
            for _ in range(32):
                psum_fill = pss.tile([P, JC, G * H], f32, tag="ps")
                nc.tensor.matmul(
                    psum_fill[:, :, :],
                    lhsT=ones_sb[:, :],
                    rhs=w16_sb[:, 0, :, :, :],
                    start=True,
                    stop=True,
                )

            # ---- I. M[b] = sum_g wsum[b,g] * P_g ----
            mh = sing.tile([P, B, DC, FSL], f16)
            for b in range(B):
                nc.vector.tensor_scalar(
                    out=m16[:, b, :, :],
                    in0=p16[:, 0, :, :],
                    scalar1=wsum_bc[:, b, 0:1],
                    scalar2=None,
                    op0=mybir.AluOpType.mult,
                )
                nc.vector.tensor_scalar(
                    out=mh[:, b, :, :],
                    in0=p16[:, 4, :, :],
                    scalar1=wsum_bc[:, b, 4:5],
                    scalar2=None,
                    op0=mybir.AluOpType.mult,
                )
                for g in (1, 2, 3):
                    nc.vector.scalar_tensor_tensor(
                        out=m16[:, b, :, :],
                        in0=p16[:, g, :, :],
                        scalar=wsum_bc[:, b, g:g + 1],
                        in1=m16[:, b, :, :],
                        op0=mybir.AluOpType.mult,
                        op1=mybir.AluOpType.add,
                    )
                    nc.vector.scalar_tensor_tensor(
                        out=mh[:, b, :, :],
                        in0=p16[:, g + 4, :, :],
                        scalar=wsum_bc[:, b, g + 4:g + 5],
                        in1=mh[:, b, :, :],
                        op0=mybir.AluOpType.mult,
                        op1=mybir.AluOpType.add,
                    )
                nc.vector.tensor_tensor(
                    out=m16[:, b, :, :],
                    in0=m16[:, b, :, :],
                    in1=mh[:, b, :, :],
                    op=mybir.AluOpType.add,
                )

            # ---- K. out[b, s, fsl] = x[b] @ M[b] + cvec  (s on partitions) ----
            for b in range(B):
                for hf in range(2):
                    psum_o = ppo.tile([P, 8, FSL], f32, tag="po")
                    for jj in range(8):
                        j = hf * 8 + jj
                        for dc in range(DC):
                            nc.tensor.matmul(
                                psum_o[:, jj, :],
                                lhsT=x_sb[:, dc, b, j * P:(j + 1) * P],
                                rhs=m16[:, b, dc, :],
                                start=(dc == 0),
                                stop=(dc == DC - 1),
                            )
                    if hf == 0:
                        nc.scalar.activation(
                            out=out_sb[:, hf * 8:(hf + 1) * 8, b, :],
                            in_=psum_o[:, :, :],
                            func=mybir.ActivationFunctionType.Identity,
                        )
                    else:
                        nc.vector.tensor_copy(
                            out_sb[:, hf * 8:(hf + 1) * 8, b, :],
                            psum_o[:, :, :],
                        )
                    nc.sync.dma_start(
                        out=out_d[b, hf * 8:(hf + 1) * 8, :, :].rearrange(
                            "j p f -> p j f"
                        ),
                        in_=out_sb[:, hf * 8:(hf + 1) * 8, b, :],
                    )

            # ---- J. cvec[b] = sum_g wsum[b,g]*bvo[:,g] + bo; flip to [1,(b f)] ----
            for b in range(B):
                nc.vector.scalar_tensor_tensor(
                    out=cvec_sb[:, b:b + 1],
                    in0=bvo_sb[:, 0:1],
                    scalar=wsum_bc[0:FSL, b, 0:1],
                    in1=bo_sb[:, :],
                    op0=mybir.AluOpType.mult,
                    op1=mybir.AluOpType.add,
                )
                for g in range(1, G):
                    nc.vector.scalar_tensor_tensor(
                        out=cvec_sb[:, b:b + 1],
                        in0=bvo_sb[:, g:g + 1],
                        scalar=wsum_bc[0:FSL, b, g:g + 1],
                        in1=cvec_sb[:, b:b + 1],
                        op0=mybir.AluOpType.mult,
                        op1=mybir.AluOpType.add,
                    )
            nc.sync.dma_start(out=cv_d[:, :], in_=cvec_sb[:, :])


    nc.compile()
    return nc


def kernel(x, Wq, bq, Wk, bk, Wv, bv, Wo, bo):
    from concourse.bass_utils import run_bass_kernel_spmd

    if "nc" not in _cache:
        _cache["nc"] = _build_nc()
    nc = _cache["nc"]

    f16 = np.float16
    xT16 = np.ascontiguousarray(
        np.asarray(x, np.float32).transpose(2, 0, 1)).astype(f16)  # [d,b,s]
    wq_r = np.asarray(Wq, np.float32).reshape(D, G, H, D)
    wvT16 = np.ascontiguousarray(
        np.asarray(Wv, np.float32).reshape(D, G, D).transpose(1, 2, 0)
    ).astype(f16)                                                   # [g,e,d]
    wo_r = np.asarray(Wo, np.float32).reshape(G, D, D)
    bq_r = np.asarray(bq, np.float32).reshape(G, H, D)
    in_maps = []
    for c in range(N_CORES):
        fs = slice(c * FSL, (c + 1) * FSL)
        in_maps.append({
            "xT16": xT16,
            "wk16": np.ascontiguousarray(
                np.asarray(Wk, np.float32)[:, c * D:(c + 1) * D]).astype(f16),
            "wq16": np.ascontiguousarray(
                wq_r[:, c].transpose(2, 1, 0)).astype(
                    __import__("ml_dtypes").float8_e4m3),            # [a,h,e]
            "wvT16": wvT16,
            "wo16": np.ascontiguousarray(
                wo_r[:, :, fs].transpose(1, 0, 2)).astype(f16),      # [e,g,f]
            "bq16": np.ascontiguousarray(bq_r[c].T).astype(f16),     # [k,h]
            "bk32": np.ascontiguousarray(
                np.asarray(bk, np.float32)[c * D:(c + 1) * D]),
            "bv32": np.ascontiguousarray(
                np.asarray(bv, np.float32).reshape(G, D).T),         # [e,g]
            "bo32": np.ascontiguousarray(np.asarray(bo, np.float32)[fs]),
        })
    res = run_bass_kernel_spmd(nc, in_maps, core_ids=list(range(N_CORES)))
    _cache["last_results"] = res
    full = np.concatenate(
        [r["out16"].reshape(B, S, FSL) for r in res.results], axis=2
    ).astype(np.float32)                              # [B, S, D]
    cvec = np.concatenate(
        [r["cvec32"].T for r in res.results], axis=1
    )                                                 # [B, D]
    return full + cvec[:, None, :]
